# revision 50
# baseline (speedup 1.0000x reference)
"""Distributed Bass kernel for nn_Interaction_GraphConvolution.

Math (reference):
    x  = node_features @ linear_w.T + linear_b          [N, IN_F]
    wf = x @ weight                                     [N, C]
    G  = mask_father[:,0,:].T @ adjacency               [N, N]
    P  = G * mask_hadamard[:,0,:].T                     [N, N]
    out[c, j] = wf[j,c] * (P @ wf)[j,c] / neighbor_count[c]^2

Key factorization (sym fast path): wf = nf @ M2 + 1*bw with
M2 = lw.T @ W (host, weights only), so
    P @ wf = (P @ nf) @ M2 + (P @ 1) x bw
turning the [N,N]@[N,C] GEMM (17.2 GF/core) into two skinny GEMMs
through the rank-512 bottleneck (2.15 GF each) plus a rank-1 term.

Sharding: output columns j (node dim) split across 8 cores, 512 each.
Two SPMD launches:
  NEFF-G: core m computes its diagonal G block via symmetric recursion,
          3 full off-diagonal row-blocks, and two 256x256 quadrants of
          the shared {m, m+4} pair block of symmetric G = A^T A
          (fp8 DoubleRow, exact on 0/1).  k-rows of A that are zero over
          the core's columns are dropped host-side (exact).
  NEFF-O: core m computes Q^T = nf^T @ PT[:,J_m] (PT rows that are
          all-zero dropped host-side), Z^T = M2^T @ Q^T, wfT[:,J_m]
          on-core, and out = Z^T*wfT*inv2 + (bw x rs)*wfT*inv2 — the
          rank-1 term runs on the scalar/DVE/GpSimd engines so the PE
          streams only the three dense GEMMs.

All DRAM operands are host-packed so each DMA moves a multi-KB
contiguous line per partition; DMA issue order puts each launch's
smallest PE-unblocking input first.
"""

import os
import sys

sys.path.insert(0, "/opt/trn_rl_repo")

import numpy as np
import ml_dtypes

from concourse import bass, bacc, mybir, tile
from concourse.bass_utils import run_bass_kernel_spmd

F32 = mybir.dt.float32
BF16 = mybir.dt.bfloat16
FP8 = mybir.dt.float8e4
DR = mybir.MatmulPerfMode.DoubleRow
IDENT = mybir.ActivationFunctionType.Identity

BF = ml_dtypes.bfloat16
F8 = ml_dtypes.float8_e4m3fn

N = 4096       # nodes (== out channels C)
F_RAW = 512    # raw feature dim
IN_F = 1024    # hidden dim
C = 4096       # out channels
M = 8          # cores
JB = N // M    # 512 output columns per core

LAST_EXEC = {}
LAST_RESULTS = {}


def _build_neffg(KB0=16, KBp=16):
    """G half via symmetry.  Core m computes, for its columns C_m:
      t=0 diag block D = G[C_m, C_m] via symmetric recursion: D[:, 0:256],
          then D[256:512, 256:384] and D[384:512, 384:512] (the rest is
          mirrored on the host) — all operands are slices of one A tile.
      t=1..3 full row-blocks (m+t)%8.
      t=4 two 256x256 quadrants of the shared {m, m+4} pair block (for
          m >= 4 the stat column halves are swapped host-side so the pair
          covers all four quadrants with no duplication).
    Contraction pruning: k-rows of A that are zero over the relevant
    column sets are dropped host-side (exact).  KB0 = 256-row k-blocks for
    t=0 (A[k, C_m] != 0); KBp = k-blocks for t=1..4 (nonzero on BOTH the
    stat and moving column sets) — each t has its own permuted stat AND
    moving copy, aligned row-by-row.

    KBp == 0 selects the shared-moving mode: t=1..4 stats are packed with
    the t0 permutation (KB0 blocks) and the moving operand is the t0 tile
    itself — 6 MB less DMA, a few more matmuls; the right trade when the
    device power-throttle squeezes DMA bandwidth.

    apm [128, (KB0+4*KBx)*1024] fp8 : slot 0 = t0 operand (stat==moving);
                                      slots 1..4 = stats, (p, kbb, h, i)
    aom [128, 4*KBp*1024] fp8       : movings for t=1..4 (absent if KBp=0)
    out g1 [5*512, JB] bf16 : t0 pieces at [0:512, 0:256], [0:256,
        256:384], [0:128, 384:512]; t1-3 full rows t*512; t4 cols 0:256
    """
    KBx = KBp if KBp > 0 else KB0
    nc = bacc.Bacc()
    apm_d = nc.dram_tensor("apm", [128, (KB0 + 4 * KBx) * 1024], FP8,
                           kind="ExternalInput")
    if KBp > 0:
        aom_d = nc.dram_tensor("aom", [128, 4 * KBp * 1024], FP8,
                               kind="ExternalInput")
    g1_d = nc.dram_tensor("g1", [5 * 512, JB], BF16, kind="ExternalOutput")

    with tile.TileContext(nc) as tc:
        with tc.tile_pool(name="const", bufs=1) as constp, \
             tc.tile_pool(name="ga", bufs=3) as gap, \
             tc.tile_pool(name="gm", bufs=3) as gmp, \
             tc.tile_pool(name="io1", bufs=4) as iop, \
             tc.tile_pool(name="psg", bufs=8, space=bass.MemorySpace.PSUM) as psgp:
            aot_t = constp.tile([128, KB0, 2, 512], FP8)
            pos = 0
            while pos < KB0:
                nk = min(2, KB0 - pos)
                nc.sync.dma_start(
                    aot_t[:, pos:pos + nk, :, :],
                    apm_d[:, pos * 1024:(pos + nk) * 1024]
                    .rearrange("p (k h j) -> p k h j", k=nk, h=2))
                pos += nk

            # t0: P1 = D[:, 0:256], P2a = D[256:512, 256:384],
            #     P2b = D[384:512, 384:512]
            psg = [psgp.tile([128, 512], F32, tag="psg", name=f"psg0_{i}")
                   for i in range(4)]
            ps2 = [psgp.tile([128, 512], F32, tag="psg", name=f"psg0b_{i}")
                   for i in range(3)]
            for kbb in range(KB0):
                st, sp = (kbb == 0), (kbb == KB0 - 1)
                for ib4 in range(4):
                    nc.tensor.matmul(
                        psg[ib4][:, 0:256],
                        aot_t[:, kbb, :, ib4 * 128:(ib4 + 1) * 128],
                        aot_t[:, kbb, :, 0:256],
                        start=st, stop=sp, perf_mode=DR)
                for i2 in range(2):
                    nc.tensor.matmul(
                        ps2[i2][:, 0:128],
                        aot_t[:, kbb, :, (2 + i2) * 128:(3 + i2) * 128],
                        aot_t[:, kbb, :, 256:384],
                        start=st, stop=sp, perf_mode=DR)
                nc.tensor.matmul(
                    ps2[2][:, 0:128],
                    aot_t[:, kbb, :, 384:512],
                    aot_t[:, kbb, :, 384:512],
                    start=st, stop=sp, perf_mode=DR)
            og1 = iop.tile([128, 4, 256], BF16, tag="og256")
            for ib4 in range(4):
                if ib4 % 2 == 0:
                    nc.scalar.activation(og1[:, ib4, :], psg[ib4][:, 0:256],
                                         IDENT, bias=0.0, scale=1.0)
                else:
                    nc.vector.tensor_copy(og1[:, ib4, :], psg[ib4][:, 0:256])
            nc.sync.dma_start(
                g1_d[0:512, 0:256].rearrange("(b p) j -> p b j", p=128),
                og1[:])
            og2 = iop.tile([128, 3, 128], BF16, tag="og128")
            for i2 in range(3):
                if i2 % 2 == 0:
                    nc.scalar.activation(og2[:, i2, :], ps2[i2][:, 0:128],
                                         IDENT, bias=0.0, scale=1.0)
                else:
                    nc.vector.tensor_copy(og2[:, i2, :], ps2[i2][:, 0:128])
            nc.sync.dma_start(
                g1_d[0:256, 256:384].rearrange("(b p) j -> p b j", p=128),
                og2[:, 0:2, :])
            nc.sync.dma_start(
                g1_d[0:128, 384:512].rearrange("(b p) j -> p b j", p=128),
                og2[:, 2:3, :])

            # t1..4: stat from apm slot t; moving from aom slot t-1, or the
            # t0 tile in shared-moving mode
            for t in range(1, 5):
                a_t = gap.tile([128, KBx, 2, 512], FP8, tag="a_t")
                sbase = (KB0 + (t - 1) * KBx) * 1024
                mbase = (t - 1) * KBx * 1024
                ao_t = aot_t if KBp == 0 else \
                    gmp.tile([128, KBx, 2, 512], FP8, tag="ao_t")
                pos = 0
                while pos < KBx:
                    nk = min(8, KBx - pos)
                    nc.sync.dma_start(
                        a_t[:, pos:pos + nk, :, :],
                        apm_d[:, sbase + pos * 1024:sbase + (pos + nk) * 1024]
                        .rearrange("p (k h i) -> p k h i", k=nk, h=2))
                    if KBp > 0:
                        nc.sync.dma_start(
                            ao_t[:, pos:pos + nk, :, :],
                            aom_d[:, mbase + pos * 1024:
                                  mbase + (pos + nk) * 1024]
                            .rearrange("p (k h i) -> p k h i", k=nk, h=2))
                    pos += nk
                psgt = [psgp.tile([128, 512], F32, tag="psg",
                                  name=f"psg{t}_{i}") for i in range(4)]
                half = 512 if t < 4 else 256
                for kbb in range(KBx):
                    for ib4 in range(4):
                        jlo = 0 if (t < 4 or ib4 < 2) else 256
                        nc.tensor.matmul(
                            psgt[ib4][:, 0:half],
                            a_t[:, kbb, :, ib4 * 128:(ib4 + 1) * 128],
                            ao_t[:, kbb, :, jlo:jlo + half],
                            start=(kbb == 0), stop=(kbb == KBx - 1),
                            perf_mode=DR)
                og = iop.tile([128, 4, half], BF16, tag=f"ogt{half}")
                for ib4 in range(4):
                    if ib4 % 2 == 0:
                        nc.scalar.activation(og[:, ib4, :],
                                             psgt[ib4][:, 0:half],
                                             IDENT, bias=0.0, scale=1.0)
                    else:
                        nc.vector.tensor_copy(og[:, ib4, :],
                                              psgt[ib4][:, 0:half])
                nc.sync.dma_start(
                    g1_d[t * 512:(t + 1) * 512, 0:half]
                    .rearrange("(b p) j -> p b j", p=128), og[:])
    nc.finalize()
    return nc


def _build_neffo(B=32):
    """Factored O phase.  B = number of 128-row i-blocks kept after the
    host drops PT rows that are all-zero for this core's columns (the
    same row permutation is applied to nf, so the contraction is exact).

    ptp [128, B*512] bf16  : PT[:, J_m] packed (p, ib, j), i = ib*128+p
    nfp [128, B*512] bf16  : nf packed (p, ib, f), i = ib*128+p
    m2  [128, 16*1024] bf16: M2 packed (p, cq, g, rb, cw), f = rb*128+p,
                             c = cq*1024 + g*512 + cw
    nfT [128, 4*512] bf16  : nf[J_m].T packed (p, rb, j), f = rb*128+p
    rsb [128, 512] f32     : rs = colsums of PT[:, J_m], replicated on all
                             partitions (the rank-1 term bw x rs runs on
                             the scalar/DVE/GpSimd engines, not the PE)
    bvi [128, 96] f32      : cols 0:32 = bw*inv2, 32:64 = inv2,
                             64:96 = bw^2*inv2, packed c = cb*128+p
    out outc [C, JB] f32
    """
    nc = bacc.Bacc()
    ptp_d = nc.dram_tensor("ptp", [128, B * 512], BF16, kind="ExternalInput")
    nfp_d = nc.dram_tensor("nfp", [128, B * 512], BF16, kind="ExternalInput")
    m2_d = nc.dram_tensor("m2", [128, 16 * 1024], BF16, kind="ExternalInput")
    nfT_d = nc.dram_tensor("nfT", [128, 4 * JB], BF16, kind="ExternalInput")
    rsb_d = nc.dram_tensor("rsb", [128, 512], F32, kind="ExternalInput")
    bvi_d = nc.dram_tensor("bvi", [128, 96], F32, kind="ExternalInput")
    out_d = nc.dram_tensor("outc", [C, JB], F32, kind="ExternalOutput")

    with tile.TileContext(nc) as tc:
        with tc.tile_pool(name="const", bufs=1) as constp, \
             tc.tile_pool(name="m2p", bufs=2) as m2p, \
             tc.tile_pool(name="wz", bufs=4) as wzp, \
             tc.tile_pool(name="oo", bufs=4) as oop, \
             tc.tile_pool(name="ps2", bufs=8, space=bass.MemorySpace.PSUM) as psp:
            ptp_t = constp.tile([128, B, 512], BF16)
            nfp_t = constp.tile([128, B, 512], BF16)
            qt_t = constp.tile([128, 4, 512], BF16)
            nfT_t = constp.tile([128, 4, JB], BF16)
            rsb_t = constp.tile([128, 512], F32)
            bvi_t = constp.tile([128, 96], F32)
            bwi_t = bvi_t[:, 0:32]
            inv_t = bvi_t[:, 32:64]
            bw2_t = bvi_t[:, 64:96]

            # critical-path DMA order: Q's first 4-block chunk (1 MB) is the
            # smallest dependency that lets the PE start; the W inputs and
            # the rest of the Q stream land behind it.
            nc.sync.dma_start(bvi_t, bvi_d[:])
            m2_t0 = m2p.tile([128, 2, 4, 512], BF16, tag="m2_t", name="m2_t0")

            def w_block(m2_t, cb, c4, g):
                pwf = psp.tile([128, 512], F32, tag="pwf", bufs=2,
                               name=f"pwf{cb}")
                for rb in range(4):
                    nc.tensor.matmul(
                        pwf[:],
                        m2_t[:, g, rb, c4 * 128:(c4 + 1) * 128],
                        nfT_t[:, rb, :],
                        start=(rb == 0), stop=(rb == 3))
                # wsb = wf*inv2 ; wsb2 = wf*bw*inv2 (for the rank-1 term)
                wsb = wzp.tile([128, 512], F32, tag="wsb", bufs=8,
                               name=f"wsb{cb}")
                nc.scalar.activation(
                    wsb[:], pwf[:], IDENT,
                    bias=bwi_t[:, cb:cb + 1], scale=inv_t[:, cb:cb + 1])
                wsb2 = wzp.tile([128, 512], F32, tag="wsb2", bufs=8,
                                name=f"wsb2{cb}")
                nc.scalar.activation(
                    wsb2[:], pwf[:], IDENT,
                    bias=bw2_t[:, cb:cb + 1], scale=bwi_t[:, cb:cb + 1])
                return wsb, wsb2

            def z_block(m2_t, ws, osb, cb, c4, g):
                wsb, wsb2 = ws
                pz = psp.tile([128, 512], F32, tag="pz", bufs=2,
                              name=f"pz{cb}")
                for rb in range(4):
                    nc.tensor.matmul(
                        pz[:],
                        m2_t[:, g, rb, c4 * 128:(c4 + 1) * 128],
                        qt_t[:, rb, :],
                        start=(rb == 0), stop=(rb == 3))
                # out = pz*wsb + (bw x rs)*wf*inv2, rank-1 term off the PE
                t1 = wzp.tile([128, 512], F32, tag="t1", bufs=4,
                              name=f"t1{cb}")
                nc.vector.tensor_mul(t1[:], pz[:], wsb[:])
                t2 = wzp.tile([128, 512], F32, tag="t2", bufs=4,
                              name=f"t2{cb}")
                nc.gpsimd.tensor_mul(t2[:], wsb2[:], rsb_t[:])
                nc.gpsimd.tensor_add(osb[:, c4, :], t1[:], t2[:])

            def q_dma(pos, nb):
                nc.sync.dma_start(
                    nfp_t[:, pos:pos + nb, :],
                    nfp_d[:, pos * 512:(pos + nb) * 512]
                    .rearrange("p (b f) -> p b f", b=nb))
                nc.sync.dma_start(
                    ptp_t[:, pos:pos + nb, :],
                    ptp_d[:, pos * 512:(pos + nb) * 512]
                    .rearrange("p (b j) -> p b j", b=nb))

            qps = [psp.tile([128, 512], F32, tag="qps", name=f"qps{i}", bufs=4)
                   for i in range(4)]

            def q_mms(lo, hi):
                # Q^T[f, j] += sum_i nf[i, f] * PT[i, j], i-blocks [lo, hi)
                for ib in range(lo, min(hi, B)):
                    for fb in range(4):
                        nc.tensor.matmul(
                            qps[fb][:],
                            nfp_t[:, ib, fb * 128:(fb + 1) * 128],
                            ptp_t[:, ib, :],
                            start=(ib == 0), stop=(ib == B - 1))

            # interleave: Q chunks first (smallest PE-unblocking data), W g0
            # and g1 slotted in as their inputs land.  The W accumulation
            # groups use separate PSUM banks, so they nest freely inside
            # Q's open accumulation chain.
            wsb0 = [[None] * 4 for _ in range(2)]
            q_dma(0, min(2, B))
            if B > 2:
                q_dma(2, min(2, B - 2))
            nc.sync.dma_start(bvi_t, bvi_d[:])
            q_mms(0, 4)
            nc.sync.dma_start(nfT_t[:],
                              nfT_d[:].rearrange("p (r j) -> p r j", r=4))
            nc.sync.dma_start(
                m2_t0[:, 0],
                m2_d[:, 0:2048].rearrange("p (r c) -> p r c", r=4))
            if B > 4:
                q_dma(4, min(4, B - 4))
            for c4 in range(4):
                wsb0[0][c4] = w_block(m2_t0, c4, c4, 0)
            nc.sync.dma_start(
                m2_t0[:, 1],
                m2_d[:, 2048:4096].rearrange("p (r c) -> p r c", r=4))
            q_mms(4, 8)
            if B > 8:
                q_dma(8, min(8, B - 8))
            for c4 in range(4):
                wsb0[1][c4] = w_block(m2_t0, 4 + c4, c4, 1)
            q_mms(8, 16)
            pos = 16
            while pos < B:
                nb = min(8, B - pos)
                q_dma(pos, nb)
                q_mms(pos, pos + nb)
                pos += nb
            nc.sync.dma_start(rsb_t[:], rsb_d[:])
            for fb in range(4):
                if fb % 2 == 0:
                    nc.scalar.activation(qt_t[:, fb, :], qps[fb][:],
                                         IDENT, bias=0.0, scale=1.0)
                else:
                    nc.vector.tensor_copy(qt_t[:, fb, :], qps[fb][:])

            # Z for cq0 using the stored wsb tiles
            for g in range(2):
                osb = oop.tile([128, 4, 512], F32, tag="osb", name=f"osb0{g}")
                for c4 in range(4):
                    z_block(m2_t0, wsb0[g][c4], osb, g * 4 + c4, c4, g)
                    if c4 % 2 == 1:
                        base = g * 512 + (c4 - 1) * 128
                        nc.sync.dma_start(
                            out_d[base:base + 256, :]
                            .rearrange("(b p) j -> p b j", p=128),
                            osb[:, c4 - 1:c4 + 1, :])

            # remaining c-quarters: W+Z interleaved per 128-c block
            for cq in range(1, 4):
                m2_t = m2p.tile([128, 2, 4, 512], BF16, tag="m2_t",
                                name=f"m2_t{cq}")
                for g in range(2):
                    nc.sync.dma_start(
                        m2_t[:, g],
                        m2_d[:, cq * 4096 + g * 2048:cq * 4096 + (g + 1) * 2048]
                        .rearrange("p (r c) -> p r c", r=4))
                for g in range(2):
                    osb = oop.tile([128, 4, 512], F32, tag="osb",
                                   name=f"osb{cq}{g}")
                    for c4 in range(4):
                        cb = cq * 8 + g * 4 + c4
                        wsb = w_block(m2_t, cb, c4, g)
                        z_block(m2_t, wsb, osb, cb, c4, g)
                        if c4 % 2 == 1:
                            base = cq * 1024 + g * 512 + (c4 - 1) * 128
                            nc.sync.dma_start(
                                out_d[base:base + 256, :]
                                .rearrange("(b p) j -> p b j", p=128),
                                osb[:, c4 - 1:c4 + 1, :])
    nc.finalize()
    return nc


def _build_neff1():
    """Non-sym fallback.  Per core: wfT[:, J_m] = M2.T @ nf[J_m].T + bw."""
    nc = bacc.Bacc()
    m2_d = nc.dram_tensor("m2", [128, 16 * 1024], BF16, kind="ExternalInput")
    nfT_d = nc.dram_tensor("nfT", [128, 4 * JB], BF16, kind="ExternalInput")
    bw_d = nc.dram_tensor("bw", [128, 32], F32, kind="ExternalInput")
    wfT_d = nc.dram_tensor("wft_rows", [C, JB], BF16, kind="ExternalOutput")

    NRB = 4
    NCQ = 4

    with tile.TileContext(nc) as tc:
        with tc.tile_pool(name="const", bufs=1) as constp, \
             tc.tile_pool(name="m2p", bufs=2) as m2p, \
             tc.tile_pool(name="ps1", bufs=8, space=bass.MemorySpace.PSUM) as psp, \
             tc.tile_pool(name="io1", bufs=4) as iop:
            nfT_t = constp.tile([128, NRB, JB], BF16)
            nc.sync.dma_start(
                nfT_t[:], nfT_d[:].rearrange("p (r j) -> p r j", r=NRB))
            bw_t = constp.tile([128, 32], F32)
            nc.sync.dma_start(bw_t[:], bw_d[:])

            for cq in range(NCQ):
                m2_t = m2p.tile([128, 2, NRB, 512], BF16, tag="m2_t")
                for g in range(2):
                    nc.sync.dma_start(
                        m2_t[:, g, :, :],
                        m2_d[:, cq * 4096 + g * 2048:cq * 4096 + (g + 1) * 2048]
                        .rearrange("p (r c) -> p r c", r=NRB))
                for g in range(2):
                    o_sb = iop.tile([128, 4, 512], BF16, tag="o_sb")
                    for c4 in range(4):
                        cb = cq * 8 + g * 4 + c4
                        pw = psp.tile([128, 512], F32, tag="pw")
                        for rb in range(NRB):
                            nc.tensor.matmul(
                                pw[:],
                                m2_t[:, g, rb, c4 * 128:(c4 + 1) * 128],
                                nfT_t[:, rb, :],
                                start=(rb == 0), stop=(rb == NRB - 1))
                        if c4 % 2 == 0:
                            nc.scalar.activation(
                                o_sb[:, c4, :], pw[:], IDENT,
                                bias=bw_t[:, cb:cb + 1], scale=1.0)
                        else:
                            nc.vector.tensor_scalar_add(
                                o_sb[:, c4, :], pw[:], bw_t[:, cb:cb + 1])
                    nc.sync.dma_start(
                        wfT_d[cq * 1024 + g * 512:cq * 1024 + (g + 1) * 512, :]
                        .rearrange("(b p) j -> p b j", p=128), o_sb[:])
    nc.finalize()
    return nc


def _build_neff2():
    """Non-sym fallback.  PT cols J_m via fp8 DoubleRow, then out[:, J_m]."""
    nc = bacc.Bacc()
    ap_d = nc.dram_tensor("ap", [128, 8 * 16 * 2 * 512], FP8, kind="ExternalInput")
    aot_d = nc.dram_tensor("aot", [128, 16 * 2 * 512], FP8, kind="ExternalInput")
    sp_d = nc.dram_tensor("sp", [128, 8 * 4 * 512], BF16, kind="ExternalInput")
    wfp_d = nc.dram_tensor("wfp", [128, 8 * 32 * 512], BF16, kind="ExternalInput")
    wtp_d = nc.dram_tensor("wtp", [128, 8 * 4 * 512], F32, kind="ExternalInput")
    out_d = nc.dram_tensor("outc", [C, JB], F32, kind="ExternalOutput")

    NIS = 8
    NKBB = 16
    NCS = 8

    with tile.TileContext(nc) as tc:
        with tc.tile_pool(name="const", bufs=1) as constp, \
             tc.tile_pool(name="ga", bufs=2) as gap, \
             tc.tile_pool(name="gs", bufs=2) as gsp, \
             tc.tile_pool(name="wfpool", bufs=2) as wfpool, \
             tc.tile_pool(name="wtpool", bufs=2) as wtpool, \
             tc.tile_pool(name="oo", bufs=2) as oop:
            aot_t = constp.tile([128, NKBB, 2, 512], FP8)
            pt_t = constp.tile([128, 32, 512], BF16)

            with tc.tile_pool(name="psg", bufs=8, space=bass.MemorySpace.PSUM) as psgp:
              for isup in range(NIS):
                a_t = gap.tile([128, NKBB, 2, 512], FP8, tag="a_t")
                if isup == 0:
                    for q in range(4):
                        nc.sync.dma_start(
                            aot_t[:, q * 4:(q + 1) * 4, :, :],
                            aot_d[:, q * 4096:(q + 1) * 4096]
                            .rearrange("p (k h j) -> p k h j", k=4, h=2))
                        nc.sync.dma_start(
                            a_t[:, q * 4:(q + 1) * 4, :, :],
                            ap_d[:, q * 4096:(q + 1) * 4096]
                            .rearrange("p (k h i) -> p k h i", k=4, h=2))
                else:
                    for hf in range(2):
                        nc.sync.dma_start(
                            a_t[:, hf * 8:(hf + 1) * 8, :, :],
                            ap_d[:, isup * 16384 + hf * 8192:
                                 isup * 16384 + (hf + 1) * 8192]
                            .rearrange("p (k h i) -> p k h i", k=8, h=2))
                s_t = gsp.tile([128, 4, 512], BF16, tag="s_t")
                nc.sync.dma_start(
                    s_t[:],
                    sp_d[:, isup * 2048:(isup + 1) * 2048]
                    .rearrange("p (b j) -> p b j", b=4))
                psg = [psgp.tile([128, 512], F32, tag="psg", name=f"psg{i}")
                       for i in range(4)]
                for kbb in range(NKBB):
                    for ib4 in range(4):
                        nc.tensor.matmul(
                            psg[ib4][:],
                            a_t[:, kbb, :, ib4 * 128:(ib4 + 1) * 128],
                            aot_t[:, kbb, :, :],
                            start=(kbb == 0), stop=(kbb == NKBB - 1),
                            perf_mode=DR)
                for ib4 in range(4):
                    nc.vector.tensor_mul(
                        pt_t[:, isup * 4 + ib4, :], psg[ib4][:], s_t[:, ib4, :])

            with tc.tile_pool(name="pso", bufs=8, space=bass.MemorySpace.PSUM) as psop:
              for csup in range(NCS):
                wf_t = wfpool.tile([128, 32, 512], BF16, tag="wf_t")
                nc.sync.dma_start(
                    wf_t[:],
                    wfp_d[:, csup * 16384:(csup + 1) * 16384]
                    .rearrange("p (b c) -> p b c", b=32))
                wt_t = wtpool.tile([128, 4, 512], F32, tag="wt_t")
                nc.sync.dma_start(
                    wt_t[:],
                    wtp_d[:, csup * 2048:(csup + 1) * 2048]
                    .rearrange("p (b j) -> p b j", b=4))
                pso = [psop.tile([128, 512], F32, tag="pso", name=f"pso{i}")
                       for i in range(4)]
                for ib in range(32):
                    for cb in range(4):
                        nc.tensor.matmul(
                            pso[cb][:],
                            wf_t[:, ib, cb * 128:(cb + 1) * 128],
                            pt_t[:, ib, :],
                            start=(ib == 0), stop=(ib == 31))
                for half in range(2):
                    o_sb = oop.tile([128, 2, 512], F32, tag="o_sb")
                    for c2 in range(2):
                        cb = half * 2 + c2
                        nc.vector.tensor_mul(
                            o_sb[:, c2, :], pso[cb][:], wt_t[:, cb, :])
                    nc.sync.dma_start(
                        out_d[csup * 512 + half * 256:
                              csup * 512 + (half + 1) * 256, :]
                        .rearrange("(b p) j -> p b j", p=128), o_sb[:])
    nc.finalize()
    return nc


# ---- host-side packing helpers ----

def _pack_m2_bw(lw, lb, W):
    M2 = (lw.T @ W).astype(np.float32)          # [F_RAW, C]
    bw = (lb.astype(np.float64) @ W.astype(np.float64)).astype(np.float32)
    m2p = np.ascontiguousarray(
        M2.reshape(4, 128, 4, 2, 512).transpose(1, 2, 3, 0, 4).reshape(128, -1)
        .astype(BF))
    return m2p, bw


def _pack_nfT(nf, m):
    nfT = nf[m * JB:(m + 1) * JB, :].T  # [F_RAW, JB]
    return np.ascontiguousarray(
        nfT.reshape(4, 128, JB).transpose(1, 0, 2).reshape(128, -1).astype(BF))


def _pack_a_fp8(A):
    # (p, isup, kbb, h, i) with k = kbb*256 + h*128 + p
    a8 = A.astype(F8)
    return np.ascontiguousarray(
        a8.reshape(16, 2, 128, 8, 512).transpose(2, 3, 0, 1, 4).reshape(128, -1))


def _pack_cols_kh(X, dtype):
    # X [N, JB] -> (p, kbb, h, j) with k = kbb*256 + h*128 + p
    return np.ascontiguousarray(
        X.astype(dtype).reshape(16, 2, 128, JB).transpose(2, 0, 1, 3).reshape(128, -1))


def _pack_rows_sup(X, dtype, nsup, nb):
    # X [N, JB] -> (p, sup, b, j) with row = sup*512 + b*128 + p
    return np.ascontiguousarray(
        X.astype(dtype).reshape(nsup, nb, 128, -1).transpose(2, 0, 1, 3).reshape(128, -1))


def _pack_rows128(X):
    # X [nb*128, F] -> (p, b, f) with row = b*128 + p
    nb = X.shape[0] // 128
    return np.ascontiguousarray(
        X.reshape(nb, 128, -1).transpose(1, 0, 2).reshape(128, -1).astype(BF))


def _pack_c32(v):
    # v [4096] -> [128, 32] with c = cb*128 + p
    return np.ascontiguousarray(v.reshape(32, 128).T)


_NCS = {}


def _get_nc(name):
    if name not in _NCS:
        if name.startswith("no"):
            _NCS[name] = _build_neffo(int(name[2:]))
        elif name.startswith("ng"):
            kb0, kbp = name[2:].split("_")
            _NCS[name] = _build_neffg(int(kb0), int(kbp))
        else:
            _NCS[name] = {"n1": _build_neff1, "n2": _build_neff2}[name]()
    return _NCS[name]


def _ensure_trace_hook():
    """Best-effort NTFF profiling shim (test harness only; grading runs
    without tracing)."""
    try:
        from antenv.axon_hooks import get_axon_ntff_profile_hook
        return get_axon_ntff_profile_hook() is not None
    except ImportError:
        pass
    try:
        import types
        if "/root/.axon_site" not in sys.path:
            sys.path.insert(0, "/root/.axon_site")
        from trn_agent_boot.trn_boot import _ntff_profile_via_ctypes
        hook = _ntff_profile_via_ctypes("/opt/axon/libaxon_pjrt.so")
        if hook is None:
            return False
        import antenv
        mod = types.ModuleType("antenv.axon_hooks")
        mod.get_axon_ntff_profile_hook = lambda: hook
        mod.set_axon_ntff_profile_hook = lambda h: None
        sys.modules["antenv.axon_hooks"] = mod
        antenv.axon_hooks = mod
        from concourse import bass_utils as _bu
        _bu.upload_artifacts = lambda tmpdir: ""
        return True
    except Exception:
        return False


def _run(nc, in_maps, cores, trace, tag):
    if trace:
        try:
            r = run_bass_kernel_spmd(nc, in_maps, cores, trace=True)
            LAST_EXEC[tag] = r.exec_time_ns
            LAST_RESULTS[tag] = r
            return r
        except Exception as e:
            print(f"trace run failed ({e!r}); retrying without trace")
    return run_bass_kernel_spmd(nc, in_maps, cores)


def kernel(node_features, adjacency_matrix, mask_father, neighbor_count,
           mask_hadamard, linear_w, linear_b, weight):
    trace = bool(int(os.environ.get("BASS_KERNEL_TRACE", "0"))) and _ensure_trace_hook()
    cores = list(range(M))

    nf = np.ascontiguousarray(np.asarray(node_features, dtype=np.float32))
    A = np.ascontiguousarray(np.asarray(adjacency_matrix, dtype=np.float32))
    Ao = np.ascontiguousarray(np.asarray(mask_father, dtype=np.float32)[:, 0, :])
    S = np.ascontiguousarray(np.asarray(mask_hadamard, dtype=np.float32)[:, 0, :])
    ncnt = np.asarray(neighbor_count, dtype=np.float32)
    lw = np.asarray(linear_w, dtype=np.float32)
    lb = np.asarray(linear_b, dtype=np.float32)
    W = np.ascontiguousarray(np.asarray(weight, dtype=np.float32))

    inv2 = (1.0 / np.square(ncnt.astype(np.float64)))[:, 0].astype(np.float32)
    m2p, bw = _pack_m2_bw(lw, lb, W)

    # mask_father == adjacency makes G = A^T A symmetric; the sym path
    # computes only 5/8 of G per core in launch 1 and mirrors on the host.
    sym = np.array_equal(Ao, A)

    if sym:
        # ---- launch 1: G blocks only ----
        a8 = A.astype(F8)
        # per-core contraction pruning: drop k-rows with A[k, C_m] == 0
        # (zero moving row => zero contribution to every block).  Shared-
        # moving mode (KBp=0): one permutation for all slots, the t0 tile
        # doubles as every moving operand — least DMA, which wins when the
        # device power-throttle squeezes bandwidth.
        pi0s = [np.nonzero(A[:, m * 512:(m + 1) * 512].any(axis=1))[0]
                for m in range(M)]
        KB0 = max(1, max((len(p) + 255) // 256 for p in pi0s))

        def _slotpack(X):
            kbx = X.shape[0] // 256
            return X.reshape(kbx, 2, 128, 512).transpose(2, 0, 1, 3) \
                    .reshape(128, -1)

        in1 = []
        for m in range(M):
            sl = slice(m * 512, (m + 1) * 512)
            pi = pi0s[m]
            stats = []
            a0 = np.zeros((KB0 * 256, 512), dtype=F8)
            a0[:len(pi)] = a8[pi, sl]
            stats.append(a0)
            for t in range(1, 5):
                bi = (m + t) % 8
                cols = np.arange(bi * 512, (bi + 1) * 512)
                if t == 4 and m >= 4:
                    cols = np.concatenate([cols[256:], cols[:256]])
                sb = np.zeros((KB0 * 256, 512), dtype=F8)
                sb[:len(pi)] = a8[np.ix_(pi, cols)]
                stats.append(sb)
            in1.append({"apm": np.ascontiguousarray(
                np.concatenate([_slotpack(s) for s in stats], axis=1))})
        r1 = _run(_get_nc(f"ng{KB0}_0"), in1, cores, trace, "neff1")

        # assemble full G from the pieces + symmetry, mask with S
        Gf = np.empty((N, N), dtype=np.float32)
        g1s = [r1.results[m]["g1"] for m in range(M)]
        for m in range(M):
            # diag block from the symmetric-recursion pieces
            D = np.empty((512, 512), dtype=np.float32)
            D[:, 0:256] = g1s[m][0:512, 0:256]
            D[0:256, 256:512] = g1s[m][256:512, 0:256].T
            D[256:512, 256:384] = g1s[m][0:256, 256:384]
            D[256:384, 384:512] = g1s[m][128:256, 256:384].T
            D[384:512, 384:512] = g1s[m][0:128, 384:512]
            Gf[m * 512:(m + 1) * 512, m * 512:(m + 1) * 512] = D
            for t in range(1, 4):
                bi = (m + t) % 8
                Gf[bi * 512:(bi + 1) * 512, m * 512:(m + 1) * 512] = \
                    g1s[m][t * 512:(t + 1) * 512, :]
        for m in range(M):
            for d in (5, 6, 7):
                bi = (m + d) % 8
                tp = (m - bi) % 8
                Gf[bi * 512:(bi + 1) * 512, m * 512:(m + 1) * 512] = \
                    g1s[bi][tp * 512:(tp + 1) * 512, :].T
        # pair blocks {p, p+4}: four 256x256 quadrants split across the pair
        for p in range(4):
            q = p + 4
            B = np.empty((512, 512), dtype=np.float32)
            B[0:256, 0:256] = g1s[p][2048:2304, 0:256]
            B[256:512, 256:512] = g1s[p][2304:2560, 0:256]
            B[0:256, 256:512] = g1s[q][2048:2304, 0:256].T
            B[256:512, 0:256] = g1s[q][2304:2560, 0:256].T
            Gf[q * 512:(q + 1) * 512, p * 512:(p + 1) * 512] = B
            Gf[p * 512:(p + 1) * 512, q * 512:(q + 1) * 512] = B.T
        pt = Gf * S  # PT[i, j] (G symmetric)

        # ---- launch 2: factored O phase ----
        # drop PT rows that are all-zero for each core's columns (exact);
        # B = max block count across cores keeps the NEFF uniform (SPMD)
        bw64 = bw.astype(np.float64)
        bvip = np.ascontiguousarray(np.concatenate(
            [_pack_c32((bw64 * inv2).astype(np.float32)),
             _pack_c32(inv2),
             _pack_c32((bw64 * bw64 * inv2).astype(np.float32))], axis=1))
        parts = []
        for m in range(M):
            sl = slice(m * JB, (m + 1) * JB)
            ptm = np.ascontiguousarray(pt[:, sl])
            rs = ptm.sum(axis=0)
            nz = np.nonzero(ptm.any(axis=1))[0]
            parts.append((ptm, rs, nz))
        B = max(1, max((len(nz) + 127) // 128 for _, _, nz in parts))
        in2 = []
        for m, (ptm, rs, nz) in enumerate(parts):
            pt_sel = np.zeros((B * 128, JB), dtype=np.float32)
            pt_sel[:len(nz)] = ptm[nz]
            nf_sel = np.zeros((B * 128, F_RAW), dtype=np.float32)
            nf_sel[:len(nz)] = nf[nz]
            in2.append({
                "ptp": _pack_rows128(pt_sel),
                "nfp": _pack_rows128(nf_sel),
                "m2": m2p,
                "nfT": _pack_nfT(nf, m),
                "rsb": np.ascontiguousarray(
                    np.tile(rs.astype(np.float32), (128, 1))),
                "bvi": bvip,
            })
        r2 = _run(_get_nc(f"no{B}"), in2, cores, trace, "neff2")
    else:
        # ---- fallback: original two-launch path ----
        bwp = _pack_c32(bw)
        in1 = [{"m2": m2p, "nfT": _pack_nfT(nf, m), "bw": bwp}
               for m in range(M)]
        r1 = _run(_get_nc("n1"), in1, cores, trace, "neff1")
        wfT = np.concatenate([r1.results[m]["wft_rows"] for m in range(M)],
                             axis=1)
        wfb = np.ascontiguousarray(wfT.T)  # [N, C] bf16
        wfT32 = wfT.astype(np.float32)
        wfp = np.ascontiguousarray(
            wfb.reshape(32, 128, 8, 512).transpose(1, 2, 0, 3).reshape(128, -1))
        a_pack = _pack_a_fp8(A)
        in2 = []
        for m in range(M):
            sl = slice(m * JB, (m + 1) * JB)
            wt = wfT32[:, sl] * inv2[:, None]
            in2.append({
                "ap": a_pack,
                "aot": _pack_cols_kh(np.ascontiguousarray(Ao[:, sl]), F8),
                "sp": _pack_rows_sup(np.ascontiguousarray(S[:, sl]), BF, 8, 4),
                "wfp": wfp,
                "wtp": _pack_rows_sup(wt.astype(np.float32), np.float32, 8, 4),
            })
        r2 = _run(_get_nc("n2"), in2, cores, trace, "neff2")

    out = np.empty((C, N), dtype=np.float32)
    for m in range(M):
        out[:, m * JB:(m + 1) * JB] = r2.results[m]["outc"]
    return out


# revision 51
# speedup vs baseline: 1.0780x; 1.0780x over previous
"""Distributed Bass kernel for nn_Interaction_GraphConvolution.

Math (reference):
    x  = node_features @ linear_w.T + linear_b          [N, IN_F]
    wf = x @ weight                                     [N, C]
    G  = mask_father[:,0,:].T @ adjacency               [N, N]
    P  = G * mask_hadamard[:,0,:].T                     [N, N]
    out[c, j] = wf[j,c] * (P @ wf)[j,c] / neighbor_count[c]^2

Key factorization (sym fast path): wf = nf @ M2 + 1*bw with
M2 = lw.T @ W (host, weights only), so
    P @ wf = (P @ nf) @ M2 + (P @ 1) x bw
turning the [N,N]@[N,C] GEMM (17.2 GF/core) into two skinny GEMMs
through the rank-512 bottleneck (2.15 GF each) plus a rank-1 term.

Sharding: output columns j (node dim) split across 8 cores, 512 each.
Two SPMD launches:
  NEFF-G: core m computes its diagonal G block via symmetric recursion,
          3 full off-diagonal row-blocks, and two 256x256 quadrants of
          the shared {m, m+4} pair block of symmetric G = A^T A
          (fp8 DoubleRow, exact on 0/1).  k-rows of A that are zero over
          the core's columns are dropped host-side (exact).
  NEFF-O: core m computes Q^T = nf^T @ PT[:,J_m] (PT rows that are
          all-zero dropped host-side), Z^T = M2^T @ Q^T, wfT[:,J_m]
          on-core, and out = Z^T*wfT*inv2 + (bw x rs)*wfT*inv2 — the
          rank-1 term runs on the scalar/DVE/GpSimd engines so the PE
          streams only the three dense GEMMs.

All DRAM operands are host-packed so each DMA moves a multi-KB
contiguous line per partition; DMA issue order puts each launch's
smallest PE-unblocking input first.
"""

import os
import sys

sys.path.insert(0, "/opt/trn_rl_repo")

import numpy as np
import ml_dtypes

from concourse import bass, bacc, mybir, tile
from concourse.bass_utils import run_bass_kernel_spmd

F32 = mybir.dt.float32
BF16 = mybir.dt.bfloat16
FP8 = mybir.dt.float8e4
DR = mybir.MatmulPerfMode.DoubleRow
IDENT = mybir.ActivationFunctionType.Identity

BF = ml_dtypes.bfloat16
F8 = ml_dtypes.float8_e4m3fn

N = 4096       # nodes (== out channels C)
F_RAW = 512    # raw feature dim
IN_F = 1024    # hidden dim
C = 4096       # out channels
M = 8          # cores
JB = N // M    # 512 output columns per core

LAST_EXEC = {}
LAST_RESULTS = {}


def _build_neffg(KB0=16, KBp=16):
    """G half via symmetry.  Core m computes, for its columns C_m:
      t=0 diag block D = G[C_m, C_m] via symmetric recursion: D[:, 0:256],
          then D[256:512, 256:384] and D[384:512, 384:512] (the rest is
          mirrored on the host) — all operands are slices of one A tile.
      t=1..3 full row-blocks (m+t)%8.
      t=4 two 256x256 quadrants of the shared {m, m+4} pair block (for
          m >= 4 the stat column halves are swapped host-side so the pair
          covers all four quadrants with no duplication).
    Contraction pruning: k-rows of A that are zero over the relevant
    column sets are dropped host-side (exact).  KB0 = 256-row k-blocks for
    t=0 (A[k, C_m] != 0); KBp = k-blocks for t=1..4 (nonzero on BOTH the
    stat and moving column sets) — each t has its own permuted stat AND
    moving copy, aligned row-by-row.

    KBp == 0 selects the shared-moving mode: t=1..4 stats are packed with
    the t0 permutation (KB0 blocks) and the moving operand is the t0 tile
    itself — 6 MB less DMA, a few more matmuls; the right trade when the
    device power-throttle squeezes DMA bandwidth.

    apm [128, (KB0+4*KBx)*1024] fp8 : slot 0 = t0 operand (stat==moving);
                                      slots 1..4 = stats, (p, kbb, h, i)
    aom [128, 4*KBp*1024] fp8       : movings for t=1..4 (absent if KBp=0)
    out g1 [5*512, JB] bf16 : t0 pieces at [0:512, 0:256], [0:256,
        256:384], [0:128, 384:512]; t1-3 full rows t*512; t4 cols 0:256
    """
    KBx = KBp if KBp > 0 else KB0
    nc = bacc.Bacc()
    apm_d = nc.dram_tensor("apm", [128, (KB0 + 4 * KBx) * 1024], FP8,
                           kind="ExternalInput")
    if KBp > 0:
        aom_d = nc.dram_tensor("aom", [128, 4 * KBp * 1024], FP8,
                               kind="ExternalInput")
    g1_d = nc.dram_tensor("g1", [5 * 512, JB], BF16, kind="ExternalOutput")

    with tile.TileContext(nc) as tc:
        with tc.tile_pool(name="const", bufs=1) as constp, \
             tc.tile_pool(name="ga", bufs=3) as gap, \
             tc.tile_pool(name="gm", bufs=3) as gmp, \
             tc.tile_pool(name="io1", bufs=4) as iop, \
             tc.tile_pool(name="psg", bufs=8, space=bass.MemorySpace.PSUM) as psgp:
            aot_t = constp.tile([128, KB0, 2, 512], FP8)
            pos = 0
            while pos < KB0:
                nk = min(2, KB0 - pos)
                nc.sync.dma_start(
                    aot_t[:, pos:pos + nk, :, :],
                    apm_d[:, pos * 1024:(pos + nk) * 1024]
                    .rearrange("p (k h j) -> p k h j", k=nk, h=2))
                pos += nk

            # t0: P1 = D[:, 0:256], P2a = D[256:512, 256:384],
            #     P2b = D[384:512, 384:512]
            psg = [psgp.tile([128, 512], F32, tag="psg", name=f"psg0_{i}")
                   for i in range(4)]
            ps2 = [psgp.tile([128, 512], F32, tag="psg", name=f"psg0b_{i}")
                   for i in range(3)]
            for kbb in range(KB0):
                st, sp = (kbb == 0), (kbb == KB0 - 1)
                for ib4 in range(4):
                    nc.tensor.matmul(
                        psg[ib4][:, 0:256],
                        aot_t[:, kbb, :, ib4 * 128:(ib4 + 1) * 128],
                        aot_t[:, kbb, :, 0:256],
                        start=st, stop=sp, perf_mode=DR)
                for i2 in range(2):
                    nc.tensor.matmul(
                        ps2[i2][:, 0:128],
                        aot_t[:, kbb, :, (2 + i2) * 128:(3 + i2) * 128],
                        aot_t[:, kbb, :, 256:384],
                        start=st, stop=sp, perf_mode=DR)
                nc.tensor.matmul(
                    ps2[2][:, 0:128],
                    aot_t[:, kbb, :, 384:512],
                    aot_t[:, kbb, :, 384:512],
                    start=st, stop=sp, perf_mode=DR)
            og1 = iop.tile([128, 4, 256], BF16, tag="og256")
            for ib4 in range(4):
                if ib4 % 2 == 0:
                    nc.scalar.activation(og1[:, ib4, :], psg[ib4][:, 0:256],
                                         IDENT, bias=0.0, scale=1.0)
                else:
                    nc.vector.tensor_copy(og1[:, ib4, :], psg[ib4][:, 0:256])
            nc.sync.dma_start(
                g1_d[0:512, 0:256].rearrange("(b p) j -> p b j", p=128),
                og1[:])
            og2 = iop.tile([128, 3, 128], BF16, tag="og128")
            for i2 in range(3):
                if i2 % 2 == 0:
                    nc.scalar.activation(og2[:, i2, :], ps2[i2][:, 0:128],
                                         IDENT, bias=0.0, scale=1.0)
                else:
                    nc.vector.tensor_copy(og2[:, i2, :], ps2[i2][:, 0:128])
            nc.sync.dma_start(
                g1_d[0:256, 256:384].rearrange("(b p) j -> p b j", p=128),
                og2[:, 0:2, :])
            nc.sync.dma_start(
                g1_d[0:128, 384:512].rearrange("(b p) j -> p b j", p=128),
                og2[:, 2:3, :])

            # t1..4: stat from apm slot t; moving from aom slot t-1, or the
            # t0 tile in shared-moving mode
            for t in range(1, 5):
                a_t = gap.tile([128, KBx, 2, 512], FP8, tag="a_t")
                sbase = (KB0 + (t - 1) * KBx) * 1024
                mbase = (t - 1) * KBx * 1024
                ao_t = aot_t if KBp == 0 else \
                    gmp.tile([128, KBx, 2, 512], FP8, tag="ao_t")
                pos = 0
                while pos < KBx:
                    nk = min(8, KBx - pos)
                    nc.sync.dma_start(
                        a_t[:, pos:pos + nk, :, :],
                        apm_d[:, sbase + pos * 1024:sbase + (pos + nk) * 1024]
                        .rearrange("p (k h i) -> p k h i", k=nk, h=2))
                    if KBp > 0:
                        nc.sync.dma_start(
                            ao_t[:, pos:pos + nk, :, :],
                            aom_d[:, mbase + pos * 1024:
                                  mbase + (pos + nk) * 1024]
                            .rearrange("p (k h i) -> p k h i", k=nk, h=2))
                    pos += nk
                psgt = [psgp.tile([128, 512], F32, tag="psg",
                                  name=f"psg{t}_{i}") for i in range(4)]
                half = 512 if t < 4 else 256
                for kbb in range(KBx):
                    for ib4 in range(4):
                        jlo = 0 if (t < 4 or ib4 < 2) else 256
                        nc.tensor.matmul(
                            psgt[ib4][:, 0:half],
                            a_t[:, kbb, :, ib4 * 128:(ib4 + 1) * 128],
                            ao_t[:, kbb, :, jlo:jlo + half],
                            start=(kbb == 0), stop=(kbb == KBx - 1),
                            perf_mode=DR)
                og = iop.tile([128, 4, half], BF16, tag=f"ogt{half}")
                for ib4 in range(4):
                    if ib4 % 2 == 0:
                        nc.scalar.activation(og[:, ib4, :],
                                             psgt[ib4][:, 0:half],
                                             IDENT, bias=0.0, scale=1.0)
                    else:
                        nc.vector.tensor_copy(og[:, ib4, :],
                                              psgt[ib4][:, 0:half])
                nc.sync.dma_start(
                    g1_d[t * 512:(t + 1) * 512, 0:half]
                    .rearrange("(b p) j -> p b j", p=128), og[:])
    nc.finalize()
    return nc


def _build_neffo(B=32):
    """Factored O phase.  B = number of 128-row i-blocks kept after the
    host drops PT rows that are all-zero for this core's columns (the
    same row permutation is applied to nf, so the contraction is exact).

    ptp [128, B*512] bf16  : PT[:, J_m] packed (p, ib, j), i = ib*128+p
    nfp [128, B*512] bf16  : nf packed (p, ib, f), i = ib*128+p
    m2  [128, 16*1024] bf16: M2 packed (p, cq, g, rb, cw), f = rb*128+p,
                             c = cq*1024 + g*512 + cw
    nfT [128, 4*512] bf16  : nf[J_m].T packed (p, rb, j), f = rb*128+p
    rsb [128, 512] f32     : rs = colsums of PT[:, J_m], replicated on all
                             partitions (the rank-1 term bw x rs runs on
                             the scalar/DVE/GpSimd engines, not the PE)
    bvi [128, 96] f32      : cols 0:32 = bw*inv2, 32:64 = inv2,
                             64:96 = bw^2*inv2, packed c = cb*128+p
    out outc [C, JB] f32
    """
    nc = bacc.Bacc()
    ptp_d = nc.dram_tensor("ptp", [128, B * 512], BF16, kind="ExternalInput")
    nfp_d = nc.dram_tensor("nfp", [128, B * 512], BF16, kind="ExternalInput")
    m2_d = nc.dram_tensor("m2", [128, 16 * 1024], BF16, kind="ExternalInput")
    nfT_d = nc.dram_tensor("nfT", [128, 4 * JB], BF16, kind="ExternalInput")
    rsb_d = nc.dram_tensor("rsb", [128, 512], F32, kind="ExternalInput")
    bvi_d = nc.dram_tensor("bvi", [128, 96], F32, kind="ExternalInput")
    out_d = nc.dram_tensor("outc", [C, JB], F32, kind="ExternalOutput")

    with tile.TileContext(nc) as tc:
        with tc.tile_pool(name="const", bufs=1) as constp, \
             tc.tile_pool(name="m2p", bufs=2) as m2p, \
             tc.tile_pool(name="wz", bufs=4) as wzp, \
             tc.tile_pool(name="oo", bufs=4) as oop, \
             tc.tile_pool(name="ps2", bufs=8, space=bass.MemorySpace.PSUM) as psp:
            ptp_t = constp.tile([128, B, 512], BF16)
            nfp_t = constp.tile([128, B, 512], BF16)
            qt_t = constp.tile([128, 4, 512], BF16)
            nfT_t = constp.tile([128, 4, JB], BF16)
            rsb_t = constp.tile([128, 512], F32)
            bvi_t = constp.tile([128, 96], F32)
            bwi_t = bvi_t[:, 0:32]
            inv_t = bvi_t[:, 32:64]
            bw2_t = bvi_t[:, 64:96]

            # critical-path DMA order: Q's first 4-block chunk (1 MB) is the
            # smallest dependency that lets the PE start; the W inputs and
            # the rest of the Q stream land behind it.
            nc.sync.dma_start(bvi_t, bvi_d[:])
            m2_t0 = m2p.tile([128, 2, 4, 512], BF16, tag="m2_t", name="m2_t0")

            def w_block(m2_t, cb, c4, g):
                pwf = psp.tile([128, 512], F32, tag="pwf", bufs=2,
                               name=f"pwf{cb}")
                for rb in range(4):
                    nc.tensor.matmul(
                        pwf[:],
                        m2_t[:, g, rb, c4 * 128:(c4 + 1) * 128],
                        nfT_t[:, rb, :],
                        start=(rb == 0), stop=(rb == 3))
                # wsb = wf*inv2 ; wsb2 = wf*bw*inv2 (for the rank-1 term)
                wsb = wzp.tile([128, 512], F32, tag="wsb", bufs=8,
                               name=f"wsb{cb}")
                nc.scalar.activation(
                    wsb[:], pwf[:], IDENT,
                    bias=bwi_t[:, cb:cb + 1], scale=inv_t[:, cb:cb + 1])
                wsb2 = wzp.tile([128, 512], F32, tag="wsb2", bufs=8,
                                name=f"wsb2{cb}")
                nc.scalar.activation(
                    wsb2[:], pwf[:], IDENT,
                    bias=bw2_t[:, cb:cb + 1], scale=bwi_t[:, cb:cb + 1])
                return wsb, wsb2

            def z_block(m2_t, ws, osb, cb, c4, g):
                wsb, wsb2 = ws
                pz = psp.tile([128, 512], F32, tag="pz", bufs=2,
                              name=f"pz{cb}")
                for rb in range(4):
                    nc.tensor.matmul(
                        pz[:],
                        m2_t[:, g, rb, c4 * 128:(c4 + 1) * 128],
                        qt_t[:, rb, :],
                        start=(rb == 0), stop=(rb == 3))
                # out = pz*wsb + (bw x rs)*wf*inv2, rank-1 term off the PE
                t1 = wzp.tile([128, 512], F32, tag="t1", bufs=4,
                              name=f"t1{cb}")
                nc.vector.tensor_mul(t1[:], pz[:], wsb[:])
                t2 = wzp.tile([128, 512], F32, tag="t2", bufs=4,
                              name=f"t2{cb}")
                nc.gpsimd.tensor_mul(t2[:], wsb2[:], rsb_t[:])
                nc.vector.tensor_add(osb[:, c4, :], t1[:], t2[:])

            def q_dma(pos, nb):
                nc.sync.dma_start(
                    nfp_t[:, pos:pos + nb, :],
                    nfp_d[:, pos * 512:(pos + nb) * 512]
                    .rearrange("p (b f) -> p b f", b=nb))
                nc.sync.dma_start(
                    ptp_t[:, pos:pos + nb, :],
                    ptp_d[:, pos * 512:(pos + nb) * 512]
                    .rearrange("p (b j) -> p b j", b=nb))

            qps = [psp.tile([128, 512], F32, tag="qps", name=f"qps{i}", bufs=4)
                   for i in range(4)]

            def q_mms(lo, hi):
                # Q^T[f, j] += sum_i nf[i, f] * PT[i, j], i-blocks [lo, hi)
                for ib in range(lo, min(hi, B)):
                    for fb in range(4):
                        nc.tensor.matmul(
                            qps[fb][:],
                            nfp_t[:, ib, fb * 128:(fb + 1) * 128],
                            ptp_t[:, ib, :],
                            start=(ib == 0), stop=(ib == B - 1))

            # interleave: Q chunks first (smallest PE-unblocking data), W g0
            # and g1 slotted in as their inputs land.  The W accumulation
            # groups use separate PSUM banks, so they nest freely inside
            # Q's open accumulation chain.
            wsb0 = [[None] * 4 for _ in range(2)]
            q_dma(0, min(2, B))
            if B > 2:
                q_dma(2, min(2, B - 2))
            nc.sync.dma_start(bvi_t, bvi_d[:])
            q_mms(0, 4)
            nc.sync.dma_start(nfT_t[:],
                              nfT_d[:].rearrange("p (r j) -> p r j", r=4))
            nc.sync.dma_start(
                m2_t0[:, 0],
                m2_d[:, 0:2048].rearrange("p (r c) -> p r c", r=4))
            if B > 4:
                q_dma(4, min(4, B - 4))
            for c4 in range(4):
                wsb0[0][c4] = w_block(m2_t0, c4, c4, 0)
            nc.sync.dma_start(
                m2_t0[:, 1],
                m2_d[:, 2048:4096].rearrange("p (r c) -> p r c", r=4))
            q_mms(4, 8)
            if B > 8:
                q_dma(8, min(8, B - 8))
            for c4 in range(4):
                wsb0[1][c4] = w_block(m2_t0, 4 + c4, c4, 1)
            q_mms(8, 16)
            pos = 16
            while pos < B:
                nb = min(8, B - pos)
                q_dma(pos, nb)
                q_mms(pos, pos + nb)
                pos += nb
            nc.sync.dma_start(rsb_t[:], rsb_d[:])
            for fb in range(4):
                if fb % 2 == 0:
                    nc.scalar.activation(qt_t[:, fb, :], qps[fb][:],
                                         IDENT, bias=0.0, scale=1.0)
                else:
                    nc.vector.tensor_copy(qt_t[:, fb, :], qps[fb][:])

            # Z for cq0 using the stored wsb tiles
            for g in range(2):
                osb = oop.tile([128, 4, 512], F32, tag="osb", name=f"osb0{g}")
                for c4 in range(4):
                    z_block(m2_t0, wsb0[g][c4], osb, g * 4 + c4, c4, g)
                    if c4 % 2 == 1:
                        base = g * 512 + (c4 - 1) * 128
                        nc.sync.dma_start(
                            out_d[base:base + 256, :]
                            .rearrange("(b p) j -> p b j", p=128),
                            osb[:, c4 - 1:c4 + 1, :])

            # remaining c-quarters: W+Z interleaved per 128-c block
            for cq in range(1, 4):
                m2_t = m2p.tile([128, 2, 4, 512], BF16, tag="m2_t",
                                name=f"m2_t{cq}")
                for g in range(2):
                    nc.sync.dma_start(
                        m2_t[:, g],
                        m2_d[:, cq * 4096 + g * 2048:cq * 4096 + (g + 1) * 2048]
                        .rearrange("p (r c) -> p r c", r=4))
                for g in range(2):
                    osb = oop.tile([128, 4, 512], F32, tag="osb",
                                   name=f"osb{cq}{g}")
                    for c4 in range(4):
                        cb = cq * 8 + g * 4 + c4
                        wsb = w_block(m2_t, cb, c4, g)
                        z_block(m2_t, wsb, osb, cb, c4, g)
                        if c4 % 2 == 1:
                            base = cq * 1024 + g * 512 + (c4 - 1) * 128
                            nc.sync.dma_start(
                                out_d[base:base + 256, :]
                                .rearrange("(b p) j -> p b j", p=128),
                                osb[:, c4 - 1:c4 + 1, :])
    nc.finalize()
    return nc


def _build_neff1():
    """Non-sym fallback.  Per core: wfT[:, J_m] = M2.T @ nf[J_m].T + bw."""
    nc = bacc.Bacc()
    m2_d = nc.dram_tensor("m2", [128, 16 * 1024], BF16, kind="ExternalInput")
    nfT_d = nc.dram_tensor("nfT", [128, 4 * JB], BF16, kind="ExternalInput")
    bw_d = nc.dram_tensor("bw", [128, 32], F32, kind="ExternalInput")
    wfT_d = nc.dram_tensor("wft_rows", [C, JB], BF16, kind="ExternalOutput")

    NRB = 4
    NCQ = 4

    with tile.TileContext(nc) as tc:
        with tc.tile_pool(name="const", bufs=1) as constp, \
             tc.tile_pool(name="m2p", bufs=2) as m2p, \
             tc.tile_pool(name="ps1", bufs=8, space=bass.MemorySpace.PSUM) as psp, \
             tc.tile_pool(name="io1", bufs=4) as iop:
            nfT_t = constp.tile([128, NRB, JB], BF16)
            nc.sync.dma_start(
                nfT_t[:], nfT_d[:].rearrange("p (r j) -> p r j", r=NRB))
            bw_t = constp.tile([128, 32], F32)
            nc.sync.dma_start(bw_t[:], bw_d[:])

            for cq in range(NCQ):
                m2_t = m2p.tile([128, 2, NRB, 512], BF16, tag="m2_t")
                for g in range(2):
                    nc.sync.dma_start(
                        m2_t[:, g, :, :],
                        m2_d[:, cq * 4096 + g * 2048:cq * 4096 + (g + 1) * 2048]
                        .rearrange("p (r c) -> p r c", r=NRB))
                for g in range(2):
                    o_sb = iop.tile([128, 4, 512], BF16, tag="o_sb")
                    for c4 in range(4):
                        cb = cq * 8 + g * 4 + c4
                        pw = psp.tile([128, 512], F32, tag="pw")
                        for rb in range(NRB):
                            nc.tensor.matmul(
                                pw[:],
                                m2_t[:, g, rb, c4 * 128:(c4 + 1) * 128],
                                nfT_t[:, rb, :],
                                start=(rb == 0), stop=(rb == NRB - 1))
                        if c4 % 2 == 0:
                            nc.scalar.activation(
                                o_sb[:, c4, :], pw[:], IDENT,
                                bias=bw_t[:, cb:cb + 1], scale=1.0)
                        else:
                            nc.vector.tensor_scalar_add(
                                o_sb[:, c4, :], pw[:], bw_t[:, cb:cb + 1])
                    nc.sync.dma_start(
                        wfT_d[cq * 1024 + g * 512:cq * 1024 + (g + 1) * 512, :]
                        .rearrange("(b p) j -> p b j", p=128), o_sb[:])
    nc.finalize()
    return nc


def _build_neff2():
    """Non-sym fallback.  PT cols J_m via fp8 DoubleRow, then out[:, J_m]."""
    nc = bacc.Bacc()
    ap_d = nc.dram_tensor("ap", [128, 8 * 16 * 2 * 512], FP8, kind="ExternalInput")
    aot_d = nc.dram_tensor("aot", [128, 16 * 2 * 512], FP8, kind="ExternalInput")
    sp_d = nc.dram_tensor("sp", [128, 8 * 4 * 512], BF16, kind="ExternalInput")
    wfp_d = nc.dram_tensor("wfp", [128, 8 * 32 * 512], BF16, kind="ExternalInput")
    wtp_d = nc.dram_tensor("wtp", [128, 8 * 4 * 512], F32, kind="ExternalInput")
    out_d = nc.dram_tensor("outc", [C, JB], F32, kind="ExternalOutput")

    NIS = 8
    NKBB = 16
    NCS = 8

    with tile.TileContext(nc) as tc:
        with tc.tile_pool(name="const", bufs=1) as constp, \
             tc.tile_pool(name="ga", bufs=2) as gap, \
             tc.tile_pool(name="gs", bufs=2) as gsp, \
             tc.tile_pool(name="wfpool", bufs=2) as wfpool, \
             tc.tile_pool(name="wtpool", bufs=2) as wtpool, \
             tc.tile_pool(name="oo", bufs=2) as oop:
            aot_t = constp.tile([128, NKBB, 2, 512], FP8)
            pt_t = constp.tile([128, 32, 512], BF16)

            with tc.tile_pool(name="psg", bufs=8, space=bass.MemorySpace.PSUM) as psgp:
              for isup in range(NIS):
                a_t = gap.tile([128, NKBB, 2, 512], FP8, tag="a_t")
                if isup == 0:
                    for q in range(4):
                        nc.sync.dma_start(
                            aot_t[:, q * 4:(q + 1) * 4, :, :],
                            aot_d[:, q * 4096:(q + 1) * 4096]
                            .rearrange("p (k h j) -> p k h j", k=4, h=2))
                        nc.sync.dma_start(
                            a_t[:, q * 4:(q + 1) * 4, :, :],
                            ap_d[:, q * 4096:(q + 1) * 4096]
                            .rearrange("p (k h i) -> p k h i", k=4, h=2))
                else:
                    for hf in range(2):
                        nc.sync.dma_start(
                            a_t[:, hf * 8:(hf + 1) * 8, :, :],
                            ap_d[:, isup * 16384 + hf * 8192:
                                 isup * 16384 + (hf + 1) * 8192]
                            .rearrange("p (k h i) -> p k h i", k=8, h=2))
                s_t = gsp.tile([128, 4, 512], BF16, tag="s_t")
                nc.sync.dma_start(
                    s_t[:],
                    sp_d[:, isup * 2048:(isup + 1) * 2048]
                    .rearrange("p (b j) -> p b j", b=4))
                psg = [psgp.tile([128, 512], F32, tag="psg", name=f"psg{i}")
                       for i in range(4)]
                for kbb in range(NKBB):
                    for ib4 in range(4):
                        nc.tensor.matmul(
                            psg[ib4][:],
                            a_t[:, kbb, :, ib4 * 128:(ib4 + 1) * 128],
                            aot_t[:, kbb, :, :],
                            start=(kbb == 0), stop=(kbb == NKBB - 1),
                            perf_mode=DR)
                for ib4 in range(4):
                    nc.vector.tensor_mul(
                        pt_t[:, isup * 4 + ib4, :], psg[ib4][:], s_t[:, ib4, :])

            with tc.tile_pool(name="pso", bufs=8, space=bass.MemorySpace.PSUM) as psop:
              for csup in range(NCS):
                wf_t = wfpool.tile([128, 32, 512], BF16, tag="wf_t")
                nc.sync.dma_start(
                    wf_t[:],
                    wfp_d[:, csup * 16384:(csup + 1) * 16384]
                    .rearrange("p (b c) -> p b c", b=32))
                wt_t = wtpool.tile([128, 4, 512], F32, tag="wt_t")
                nc.sync.dma_start(
                    wt_t[:],
                    wtp_d[:, csup * 2048:(csup + 1) * 2048]
                    .rearrange("p (b j) -> p b j", b=4))
                pso = [psop.tile([128, 512], F32, tag="pso", name=f"pso{i}")
                       for i in range(4)]
                for ib in range(32):
                    for cb in range(4):
                        nc.tensor.matmul(
                            pso[cb][:],
                            wf_t[:, ib, cb * 128:(cb + 1) * 128],
                            pt_t[:, ib, :],
                            start=(ib == 0), stop=(ib == 31))
                for half in range(2):
                    o_sb = oop.tile([128, 2, 512], F32, tag="o_sb")
                    for c2 in range(2):
                        cb = half * 2 + c2
                        nc.vector.tensor_mul(
                            o_sb[:, c2, :], pso[cb][:], wt_t[:, cb, :])
                    nc.sync.dma_start(
                        out_d[csup * 512 + half * 256:
                              csup * 512 + (half + 1) * 256, :]
                        .rearrange("(b p) j -> p b j", p=128), o_sb[:])
    nc.finalize()
    return nc


# ---- host-side packing helpers ----

def _pack_m2_bw(lw, lb, W):
    M2 = (lw.T @ W).astype(np.float32)          # [F_RAW, C]
    bw = (lb.astype(np.float64) @ W.astype(np.float64)).astype(np.float32)
    m2p = np.ascontiguousarray(
        M2.reshape(4, 128, 4, 2, 512).transpose(1, 2, 3, 0, 4).reshape(128, -1)
        .astype(BF))
    return m2p, bw


def _pack_nfT(nf, m):
    nfT = nf[m * JB:(m + 1) * JB, :].T  # [F_RAW, JB]
    return np.ascontiguousarray(
        nfT.reshape(4, 128, JB).transpose(1, 0, 2).reshape(128, -1).astype(BF))


def _pack_a_fp8(A):
    # (p, isup, kbb, h, i) with k = kbb*256 + h*128 + p
    a8 = A.astype(F8)
    return np.ascontiguousarray(
        a8.reshape(16, 2, 128, 8, 512).transpose(2, 3, 0, 1, 4).reshape(128, -1))


def _pack_cols_kh(X, dtype):
    # X [N, JB] -> (p, kbb, h, j) with k = kbb*256 + h*128 + p
    return np.ascontiguousarray(
        X.astype(dtype).reshape(16, 2, 128, JB).transpose(2, 0, 1, 3).reshape(128, -1))


def _pack_rows_sup(X, dtype, nsup, nb):
    # X [N, JB] -> (p, sup, b, j) with row = sup*512 + b*128 + p
    return np.ascontiguousarray(
        X.astype(dtype).reshape(nsup, nb, 128, -1).transpose(2, 0, 1, 3).reshape(128, -1))


def _pack_rows128(X):
    # X [nb*128, F] -> (p, b, f) with row = b*128 + p
    nb = X.shape[0] // 128
    return np.ascontiguousarray(
        X.reshape(nb, 128, -1).transpose(1, 0, 2).reshape(128, -1).astype(BF))


def _pack_c32(v):
    # v [4096] -> [128, 32] with c = cb*128 + p
    return np.ascontiguousarray(v.reshape(32, 128).T)


_NCS = {}


def _get_nc(name):
    if name not in _NCS:
        if name.startswith("no"):
            _NCS[name] = _build_neffo(int(name[2:]))
        elif name.startswith("ng"):
            kb0, kbp = name[2:].split("_")
            _NCS[name] = _build_neffg(int(kb0), int(kbp))
        else:
            _NCS[name] = {"n1": _build_neff1, "n2": _build_neff2}[name]()
    return _NCS[name]


def _ensure_trace_hook():
    """Best-effort NTFF profiling shim (test harness only; grading runs
    without tracing)."""
    try:
        from antenv.axon_hooks import get_axon_ntff_profile_hook
        return get_axon_ntff_profile_hook() is not None
    except ImportError:
        pass
    try:
        import types
        if "/root/.axon_site" not in sys.path:
            sys.path.insert(0, "/root/.axon_site")
        from trn_agent_boot.trn_boot import _ntff_profile_via_ctypes
        hook = _ntff_profile_via_ctypes("/opt/axon/libaxon_pjrt.so")
        if hook is None:
            return False
        import antenv
        mod = types.ModuleType("antenv.axon_hooks")
        mod.get_axon_ntff_profile_hook = lambda: hook
        mod.set_axon_ntff_profile_hook = lambda h: None
        sys.modules["antenv.axon_hooks"] = mod
        antenv.axon_hooks = mod
        from concourse import bass_utils as _bu
        _bu.upload_artifacts = lambda tmpdir: ""
        return True
    except Exception:
        return False


def _run(nc, in_maps, cores, trace, tag):
    if trace:
        try:
            r = run_bass_kernel_spmd(nc, in_maps, cores, trace=True)
            LAST_EXEC[tag] = r.exec_time_ns
            LAST_RESULTS[tag] = r
            return r
        except Exception as e:
            print(f"trace run failed ({e!r}); retrying without trace")
    return run_bass_kernel_spmd(nc, in_maps, cores)


def kernel(node_features, adjacency_matrix, mask_father, neighbor_count,
           mask_hadamard, linear_w, linear_b, weight):
    trace = bool(int(os.environ.get("BASS_KERNEL_TRACE", "0"))) and _ensure_trace_hook()
    cores = list(range(M))

    nf = np.ascontiguousarray(np.asarray(node_features, dtype=np.float32))
    A = np.ascontiguousarray(np.asarray(adjacency_matrix, dtype=np.float32))
    Ao = np.ascontiguousarray(np.asarray(mask_father, dtype=np.float32)[:, 0, :])
    S = np.ascontiguousarray(np.asarray(mask_hadamard, dtype=np.float32)[:, 0, :])
    ncnt = np.asarray(neighbor_count, dtype=np.float32)
    lw = np.asarray(linear_w, dtype=np.float32)
    lb = np.asarray(linear_b, dtype=np.float32)
    W = np.ascontiguousarray(np.asarray(weight, dtype=np.float32))

    inv2 = (1.0 / np.square(ncnt.astype(np.float64)))[:, 0].astype(np.float32)
    m2p, bw = _pack_m2_bw(lw, lb, W)

    # mask_father == adjacency makes G = A^T A symmetric; the sym path
    # computes only 5/8 of G per core in launch 1 and mirrors on the host.
    sym = np.array_equal(Ao, A)

    if sym:
        # ---- launch 1: G blocks only ----
        a8 = A.astype(F8)
        # per-core contraction pruning: drop k-rows with A[k, C_m] == 0
        # (zero moving row => zero contribution to every block).  Shared-
        # moving mode (KBp=0): one permutation for all slots, the t0 tile
        # doubles as every moving operand — least DMA, which wins when the
        # device power-throttle squeezes bandwidth.
        pi0s = [np.nonzero(A[:, m * 512:(m + 1) * 512].any(axis=1))[0]
                for m in range(M)]
        KB0 = max(1, max((len(p) + 255) // 256 for p in pi0s))

        def _slotpack(X):
            kbx = X.shape[0] // 256
            return X.reshape(kbx, 2, 128, 512).transpose(2, 0, 1, 3) \
                    .reshape(128, -1)

        in1 = []
        for m in range(M):
            sl = slice(m * 512, (m + 1) * 512)
            pi = pi0s[m]
            stats = []
            a0 = np.zeros((KB0 * 256, 512), dtype=F8)
            a0[:len(pi)] = a8[pi, sl]
            stats.append(a0)
            for t in range(1, 5):
                bi = (m + t) % 8
                cols = np.arange(bi * 512, (bi + 1) * 512)
                if t == 4 and m >= 4:
                    cols = np.concatenate([cols[256:], cols[:256]])
                sb = np.zeros((KB0 * 256, 512), dtype=F8)
                sb[:len(pi)] = a8[np.ix_(pi, cols)]
                stats.append(sb)
            in1.append({"apm": np.ascontiguousarray(
                np.concatenate([_slotpack(s) for s in stats], axis=1))})
        r1 = _run(_get_nc(f"ng{KB0}_0"), in1, cores, trace, "neff1")

        # assemble full G from the pieces + symmetry, mask with S
        Gf = np.empty((N, N), dtype=np.float32)
        g1s = [r1.results[m]["g1"] for m in range(M)]
        for m in range(M):
            # diag block from the symmetric-recursion pieces
            D = np.empty((512, 512), dtype=np.float32)
            D[:, 0:256] = g1s[m][0:512, 0:256]
            D[0:256, 256:512] = g1s[m][256:512, 0:256].T
            D[256:512, 256:384] = g1s[m][0:256, 256:384]
            D[256:384, 384:512] = g1s[m][128:256, 256:384].T
            D[384:512, 384:512] = g1s[m][0:128, 384:512]
            Gf[m * 512:(m + 1) * 512, m * 512:(m + 1) * 512] = D
            for t in range(1, 4):
                bi = (m + t) % 8
                Gf[bi * 512:(bi + 1) * 512, m * 512:(m + 1) * 512] = \
                    g1s[m][t * 512:(t + 1) * 512, :]
        for m in range(M):
            for d in (5, 6, 7):
                bi = (m + d) % 8
                tp = (m - bi) % 8
                Gf[bi * 512:(bi + 1) * 512, m * 512:(m + 1) * 512] = \
                    g1s[bi][tp * 512:(tp + 1) * 512, :].T
        # pair blocks {p, p+4}: four 256x256 quadrants split across the pair
        for p in range(4):
            q = p + 4
            B = np.empty((512, 512), dtype=np.float32)
            B[0:256, 0:256] = g1s[p][2048:2304, 0:256]
            B[256:512, 256:512] = g1s[p][2304:2560, 0:256]
            B[0:256, 256:512] = g1s[q][2048:2304, 0:256].T
            B[256:512, 0:256] = g1s[q][2304:2560, 0:256].T
            Gf[q * 512:(q + 1) * 512, p * 512:(p + 1) * 512] = B
            Gf[p * 512:(p + 1) * 512, q * 512:(q + 1) * 512] = B.T
        pt = Gf * S  # PT[i, j] (G symmetric)

        # ---- launch 2: factored O phase ----
        # drop PT rows that are all-zero for each core's columns (exact);
        # B = max block count across cores keeps the NEFF uniform (SPMD)
        bw64 = bw.astype(np.float64)
        bvip = np.ascontiguousarray(np.concatenate(
            [_pack_c32((bw64 * inv2).astype(np.float32)),
             _pack_c32(inv2),
             _pack_c32((bw64 * bw64 * inv2).astype(np.float32))], axis=1))
        parts = []
        for m in range(M):
            sl = slice(m * JB, (m + 1) * JB)
            ptm = np.ascontiguousarray(pt[:, sl])
            rs = ptm.sum(axis=0)
            nz = np.nonzero(ptm.any(axis=1))[0]
            parts.append((ptm, rs, nz))
        B = max(1, max((len(nz) + 127) // 128 for _, _, nz in parts))
        in2 = []
        for m, (ptm, rs, nz) in enumerate(parts):
            pt_sel = np.zeros((B * 128, JB), dtype=np.float32)
            pt_sel[:len(nz)] = ptm[nz]
            nf_sel = np.zeros((B * 128, F_RAW), dtype=np.float32)
            nf_sel[:len(nz)] = nf[nz]
            in2.append({
                "ptp": _pack_rows128(pt_sel),
                "nfp": _pack_rows128(nf_sel),
                "m2": m2p,
                "nfT": _pack_nfT(nf, m),
                "rsb": np.ascontiguousarray(
                    np.tile(rs.astype(np.float32), (128, 1))),
                "bvi": bvip,
            })
        r2 = _run(_get_nc(f"no{B}"), in2, cores, trace, "neff2")
    else:
        # ---- fallback: original two-launch path ----
        bwp = _pack_c32(bw)
        in1 = [{"m2": m2p, "nfT": _pack_nfT(nf, m), "bw": bwp}
               for m in range(M)]
        r1 = _run(_get_nc("n1"), in1, cores, trace, "neff1")
        wfT = np.concatenate([r1.results[m]["wft_rows"] for m in range(M)],
                             axis=1)
        wfb = np.ascontiguousarray(wfT.T)  # [N, C] bf16
        wfT32 = wfT.astype(np.float32)
        wfp = np.ascontiguousarray(
            wfb.reshape(32, 128, 8, 512).transpose(1, 2, 0, 3).reshape(128, -1))
        a_pack = _pack_a_fp8(A)
        in2 = []
        for m in range(M):
            sl = slice(m * JB, (m + 1) * JB)
            wt = wfT32[:, sl] * inv2[:, None]
            in2.append({
                "ap": a_pack,
                "aot": _pack_cols_kh(np.ascontiguousarray(Ao[:, sl]), F8),
                "sp": _pack_rows_sup(np.ascontiguousarray(S[:, sl]), BF, 8, 4),
                "wfp": wfp,
                "wtp": _pack_rows_sup(wt.astype(np.float32), np.float32, 8, 4),
            })
        r2 = _run(_get_nc("n2"), in2, cores, trace, "neff2")

    out = np.empty((C, N), dtype=np.float32)
    for m in range(M):
        out[:, m * JB:(m + 1) * JB] = r2.results[m]["outc"]
    return out


# revision 59
# speedup vs baseline: 1.1144x; 1.0337x over previous
"""Distributed Bass kernel for nn_Interaction_GraphConvolution.

Math (reference):
    x  = node_features @ linear_w.T + linear_b          [N, IN_F]
    wf = x @ weight                                     [N, C]
    G  = mask_father[:,0,:].T @ adjacency               [N, N]
    P  = G * mask_hadamard[:,0,:].T                     [N, N]
    out[c, j] = wf[j,c] * (P @ wf)[j,c] / neighbor_count[c]^2

Key factorization (sym fast path): wf = nf @ M2 + 1*bw with
M2 = lw.T @ W (host, weights only), so
    P @ wf = (P @ nf) @ M2 + (P @ 1) x bw
turning the [N,N]@[N,C] GEMM (17.2 GF/core) into two skinny GEMMs
through the rank-512 bottleneck (2.15 GF each) plus a rank-1 term.

Sharding: output columns j (node dim) split across 8 cores, 512 each.
Two SPMD launches:
  NEFF-G: core m computes its diagonal G block via symmetric recursion,
          3 full off-diagonal row-blocks, and two 256x256 quadrants of
          the shared {m, m+4} pair block of symmetric G = A^T A
          (fp8 DoubleRow, exact on 0/1).  k-rows of A that are zero over
          the core's columns are dropped host-side (exact).
  NEFF-O: core m computes Q^T = nf^T @ PT[:,J_m] (PT rows that are
          all-zero dropped host-side), then Z^T = M2^T @ Q^T + bw x rs
          (rs = PT colsums, 5th k-block), wfT[:,J_m] on-core, and
          out = Z^T * wfT * inv2.

All DRAM operands are host-packed so each DMA moves a multi-KB
contiguous line per partition; DMA issue order puts each launch's
smallest PE-unblocking input first.
"""

import os
import sys

sys.path.insert(0, "/opt/trn_rl_repo")

import numpy as np
import ml_dtypes

from concourse import bass, bacc, mybir, tile
from concourse.bass_utils import run_bass_kernel_spmd

F32 = mybir.dt.float32
BF16 = mybir.dt.bfloat16
FP8 = mybir.dt.float8e4
DR = mybir.MatmulPerfMode.DoubleRow
IDENT = mybir.ActivationFunctionType.Identity

BF = ml_dtypes.bfloat16
F8 = ml_dtypes.float8_e4m3fn

N = 4096       # nodes (== out channels C)
F_RAW = 512    # raw feature dim
IN_F = 1024    # hidden dim
C = 4096       # out channels
M = 8          # cores
JB = N // M    # 512 output columns per core

LAST_EXEC = {}
LAST_RESULTS = {}


def _build_neffg(KB0=16, KBp=16):
    """G half via symmetry.  Core m computes, for its columns C_m:
      t=0 diag block D = G[C_m, C_m] via symmetric recursion: D[:, 0:256],
          then D[256:512, 256:384] and D[384:512, 384:512] (the rest is
          mirrored on the host) — all operands are slices of one A tile.
      t=1..3 full row-blocks (m+t)%8.
      t=4 two 256x256 quadrants of the shared {m, m+4} pair block (for
          m >= 4 the stat column halves are swapped host-side so the pair
          covers all four quadrants with no duplication).
    Contraction pruning: k-rows of A that are zero over the relevant
    column sets are dropped host-side (exact).  KB0 = 256-row k-blocks for
    t=0 (A[k, C_m] != 0); KBp = k-blocks for t=1..4 (nonzero on BOTH the
    stat and moving column sets) — each t has its own permuted stat AND
    moving copy, aligned row-by-row.

    KBp == 0 selects the shared-moving mode: t=1..4 stats are packed with
    the t0 permutation (KB0 blocks) and the moving operand is the t0 tile
    itself — 6 MB less DMA, a few more matmuls; the right trade when the
    device power-throttle squeezes DMA bandwidth.

    apm [128, (KB0+4*KBx)*1024] fp8 : slot 0 = t0 operand (stat==moving);
                                      slots 1..4 = stats, (p, kbb, h, i)
    aom [128, 4*KBp*1024] fp8       : movings for t=1..4 (absent if KBp=0)
    out g1 [5*512, JB] bf16 : t0 pieces at [0:512, 0:256], [0:256,
        256:384], [0:128, 384:512]; t1-3 full rows t*512; t4 cols 0:256
    """
    KBx = KBp if KBp > 0 else KB0
    nc = bacc.Bacc()
    apm_d = nc.dram_tensor("apm", [128, (KB0 + 4 * KBx) * 1024], FP8,
                           kind="ExternalInput")
    if KBp > 0:
        aom_d = nc.dram_tensor("aom", [128, 4 * KBp * 1024], FP8,
                               kind="ExternalInput")
    g1_d = nc.dram_tensor("g1", [5 * 512, JB], BF16, kind="ExternalOutput")

    with tile.TileContext(nc) as tc:
        with tc.tile_pool(name="const", bufs=1) as constp, \
             tc.tile_pool(name="ga", bufs=3) as gap, \
             tc.tile_pool(name="gm", bufs=3) as gmp, \
             tc.tile_pool(name="io1", bufs=4) as iop, \
             tc.tile_pool(name="psg", bufs=8, space=bass.MemorySpace.PSUM) as psgp:
            aot_t = constp.tile([128, KB0, 2, 512], FP8)
            pos = 0
            while pos < KB0:
                nk = min(2, KB0 - pos)
                nc.sync.dma_start(
                    aot_t[:, pos:pos + nk, :, :],
                    apm_d[:, pos * 1024:(pos + nk) * 1024]
                    .rearrange("p (k h j) -> p k h j", k=nk, h=2))
                pos += nk

            # t0: P1 = D[:, 0:256], P2a = D[256:512, 256:384],
            #     P2b = D[384:512, 384:512]
            psg = [psgp.tile([128, 512], F32, tag="psg", name=f"psg0_{i}")
                   for i in range(4)]
            ps2 = [psgp.tile([128, 512], F32, tag="psg", name=f"psg0b_{i}")
                   for i in range(3)]
            for kbb in range(KB0):
                st, sp = (kbb == 0), (kbb == KB0 - 1)
                for ib4 in range(4):
                    nc.tensor.matmul(
                        psg[ib4][:, 0:256],
                        aot_t[:, kbb, :, ib4 * 128:(ib4 + 1) * 128],
                        aot_t[:, kbb, :, 0:256],
                        start=st, stop=sp, perf_mode=DR)
                for i2 in range(2):
                    nc.tensor.matmul(
                        ps2[i2][:, 0:128],
                        aot_t[:, kbb, :, (2 + i2) * 128:(3 + i2) * 128],
                        aot_t[:, kbb, :, 256:384],
                        start=st, stop=sp, perf_mode=DR)
                nc.tensor.matmul(
                    ps2[2][:, 0:128],
                    aot_t[:, kbb, :, 384:512],
                    aot_t[:, kbb, :, 384:512],
                    start=st, stop=sp, perf_mode=DR)
            og1 = iop.tile([128, 4, 256], BF16, tag="og256")
            for ib4 in range(4):
                if ib4 % 2 == 0:
                    nc.scalar.activation(og1[:, ib4, :], psg[ib4][:, 0:256],
                                         IDENT, bias=0.0, scale=1.0)
                else:
                    nc.vector.tensor_copy(og1[:, ib4, :], psg[ib4][:, 0:256])
            nc.sync.dma_start(
                g1_d[0:512, 0:256].rearrange("(b p) j -> p b j", p=128),
                og1[:])
            og2 = iop.tile([128, 3, 128], BF16, tag="og128")
            for i2 in range(3):
                if i2 % 2 == 0:
                    nc.scalar.activation(og2[:, i2, :], ps2[i2][:, 0:128],
                                         IDENT, bias=0.0, scale=1.0)
                else:
                    nc.vector.tensor_copy(og2[:, i2, :], ps2[i2][:, 0:128])
            nc.sync.dma_start(
                g1_d[0:256, 256:384].rearrange("(b p) j -> p b j", p=128),
                og2[:, 0:2, :])
            nc.sync.dma_start(
                g1_d[0:128, 384:512].rearrange("(b p) j -> p b j", p=128),
                og2[:, 2:3, :])

            # t1..4: stat from apm slot t; moving from aom slot t-1, or the
            # t0 tile in shared-moving mode
            for t in range(1, 5):
                a_t = gap.tile([128, KBx, 2, 512], FP8, tag="a_t")
                sbase = (KB0 + (t - 1) * KBx) * 1024
                mbase = (t - 1) * KBx * 1024
                ao_t = aot_t if KBp == 0 else \
                    gmp.tile([128, KBx, 2, 512], FP8, tag="ao_t")
                pos = 0
                while pos < KBx:
                    nk = min(8, KBx - pos)
                    nc.sync.dma_start(
                        a_t[:, pos:pos + nk, :, :],
                        apm_d[:, sbase + pos * 1024:sbase + (pos + nk) * 1024]
                        .rearrange("p (k h i) -> p k h i", k=nk, h=2))
                    if KBp > 0:
                        nc.sync.dma_start(
                            ao_t[:, pos:pos + nk, :, :],
                            aom_d[:, mbase + pos * 1024:
                                  mbase + (pos + nk) * 1024]
                            .rearrange("p (k h i) -> p k h i", k=nk, h=2))
                    pos += nk
                psgt = [psgp.tile([128, 512], F32, tag="psg",
                                  name=f"psg{t}_{i}") for i in range(4)]
                half = 512 if t < 4 else 256
                for kbb in range(KBx):
                    for ib4 in range(4):
                        jlo = 0 if (t < 4 or ib4 < 2) else 256
                        nc.tensor.matmul(
                            psgt[ib4][:, 0:half],
                            a_t[:, kbb, :, ib4 * 128:(ib4 + 1) * 128],
                            ao_t[:, kbb, :, jlo:jlo + half],
                            start=(kbb == 0), stop=(kbb == KBx - 1),
                            perf_mode=DR)
                og = iop.tile([128, 4, half], BF16, tag=f"ogt{half}")
                for ib4 in range(4):
                    if ib4 % 2 == 0:
                        nc.scalar.activation(og[:, ib4, :],
                                             psgt[ib4][:, 0:half],
                                             IDENT, bias=0.0, scale=1.0)
                    else:
                        nc.vector.tensor_copy(og[:, ib4, :],
                                              psgt[ib4][:, 0:half])
                nc.sync.dma_start(
                    g1_d[t * 512:(t + 1) * 512, 0:half]
                    .rearrange("(b p) j -> p b j", p=128), og[:])
    nc.finalize()
    return nc


def _build_neffo(B=32):
    """Factored O phase.  B = number of 128-row i-blocks kept after the
    host drops PT rows that are all-zero for this core's columns (the
    same row permutation is applied to nf, so the contraction is exact).

    ptp [128, B*512] bf16  : PT[:, J_m] packed (p, ib, j), i = ib*128+p
    nfp [128, B*512] bf16  : nf packed (p, ib, f), i = ib*128+p
    m2  [128, 16*1024] bf16: M2 packed (p, cq, g, rb, cw), f = rb*128+p,
                             c = cq*1024 + g*512 + cw
    nfT [128, 4*512] bf16  : nf[J_m].T packed (p, rb, j), f = rb*128+p
    bwe [128, 4096] bf16   : row 0 = bw (stat for the rank-1 term; doing
                             the rank-1 on scalar/DVE/GpSimd instead was
                             measured SLOWER — GpSimd tensor ops are
                             1270 ns per [128,512] and the cross-engine
                             chain beats the 32 matmuls it saves)
    q4  [128, 512] bf16    : row 0 = rs = colsums of PT[:, J_m]
    bvi [128, 64] f32      : cols 0:32 = bw*inv2, cols 32:64 = inv2,
                             packed c = cb*128+p
    out outc [C, JB] f32
    """
    nc = bacc.Bacc()
    ptp_d = nc.dram_tensor("ptp", [128, B * 512], BF16, kind="ExternalInput")
    nfp_d = nc.dram_tensor("nfp", [128, B * 512], BF16, kind="ExternalInput")
    m2_d = nc.dram_tensor("m2", [128, 16 * 1024], BF16, kind="ExternalInput")
    nfT_d = nc.dram_tensor("nfT", [128, 4 * JB], BF16, kind="ExternalInput")
    bwe_d = nc.dram_tensor("bwe", [128, C], BF16, kind="ExternalInput")
    q4_d = nc.dram_tensor("q4", [128, 512], BF16, kind="ExternalInput")
    bvi_d = nc.dram_tensor("bvi", [128, 64], F32, kind="ExternalInput")
    out_d = nc.dram_tensor("outc", [C, JB], F32, kind="ExternalOutput")

    with tile.TileContext(nc) as tc:
        with tc.tile_pool(name="const", bufs=1) as constp, \
             tc.tile_pool(name="m2p", bufs=2) as m2p, \
             tc.tile_pool(name="wz", bufs=4) as wzp, \
             tc.tile_pool(name="oo", bufs=4) as oop, \
             tc.tile_pool(name="ps2", bufs=8, space=bass.MemorySpace.PSUM) as psp:
            ptp_t = constp.tile([128, B, 512], BF16)
            nfp_t = constp.tile([128, B, 512], BF16)
            qt_t = constp.tile([128, 5, 512], BF16)
            nfT_t = constp.tile([128, 4, JB], BF16)
            bwe_t = constp.tile([128, C], BF16)
            bvi_t = constp.tile([128, 64], F32)
            bwi_t = bvi_t[:, 0:32]
            inv_t = bvi_t[:, 32:64]

            # critical-path DMA order: Q's first 4-block chunk (1 MB) is the
            # smallest dependency that lets the PE start; the W inputs and
            # the rest of the Q stream land behind it.
            nc.sync.dma_start(bvi_t, bvi_d[:])
            m2_t0 = m2p.tile([128, 2, 4, 512], BF16, tag="m2_t", name="m2_t0")

            def w_block(m2_t, cb, c4, g):
                pwf = psp.tile([128, 512], F32, tag="pwf", bufs=2,
                               name=f"pwf{cb}")
                for rb in range(4):
                    nc.tensor.matmul(
                        pwf[:],
                        m2_t[:, g, rb, c4 * 128:(c4 + 1) * 128],
                        nfT_t[:, rb, :],
                        start=(rb == 0), stop=(rb == 3))
                wsb = wzp.tile([128, 512], F32, tag="wsb", bufs=8,
                               name=f"wsb{cb}")
                nc.scalar.activation(
                    wsb[:], pwf[:], IDENT,
                    bias=bwi_t[:, cb:cb + 1], scale=inv_t[:, cb:cb + 1])
                return wsb

            def z_block(m2_t, wsb, osb, cb, c4, g):
                pz = psp.tile([128, 512], F32, tag="pz", bufs=2,
                              name=f"pz{cb}")
                for rb in range(4):
                    nc.tensor.matmul(
                        pz[:],
                        m2_t[:, g, rb, c4 * 128:(c4 + 1) * 128],
                        qt_t[:, rb, :],
                        start=(rb == 0), stop=False)
                nc.tensor.matmul(
                    pz[:],
                    bwe_t[:, cb * 128:(cb + 1) * 128],
                    qt_t[:, 4, :],
                    start=False, stop=True)
                nc.vector.tensor_mul(osb[:, c4, :], pz[:], wsb[:])

            def q_dma(pos, nb):
                nc.sync.dma_start(
                    nfp_t[:, pos:pos + nb, :],
                    nfp_d[:, pos * 512:(pos + nb) * 512]
                    .rearrange("p (b f) -> p b f", b=nb))
                nc.sync.dma_start(
                    ptp_t[:, pos:pos + nb, :],
                    ptp_d[:, pos * 512:(pos + nb) * 512]
                    .rearrange("p (b j) -> p b j", b=nb))

            qps = [psp.tile([128, 512], F32, tag="qps", name=f"qps{i}", bufs=4)
                   for i in range(4)]

            def q_mms(lo, hi):
                # Q^T[f, j] += sum_i nf[i, f] * PT[i, j], i-blocks [lo, hi)
                for ib in range(lo, min(hi, B)):
                    for fb in range(4):
                        nc.tensor.matmul(
                            qps[fb][:],
                            nfp_t[:, ib, fb * 128:(fb + 1) * 128],
                            ptp_t[:, ib, :],
                            start=(ib == 0), stop=(ib == B - 1))

            # interleave: Q chunks first (smallest PE-unblocking data), W g0
            # and g1 slotted in as their inputs land.  The W accumulation
            # groups use separate PSUM banks, so they nest freely inside
            # Q's open accumulation chain.
            wsb0 = [[None] * 4 for _ in range(2)]
            q_dma(0, min(2, B))
            if B > 2:
                q_dma(2, min(2, B - 2))
            nc.sync.dma_start(bvi_t, bvi_d[:])
            q_mms(0, 4)
            nc.sync.dma_start(nfT_t[:],
                              nfT_d[:].rearrange("p (r j) -> p r j", r=4))
            nc.sync.dma_start(
                m2_t0[:, 0],
                m2_d[:, 0:2048].rearrange("p (r c) -> p r c", r=4))
            if B > 4:
                q_dma(4, min(4, B - 4))
            for c4 in range(4):
                wsb0[0][c4] = w_block(m2_t0, c4, c4, 0)
            nc.sync.dma_start(
                m2_t0[:, 1],
                m2_d[:, 2048:4096].rearrange("p (r c) -> p r c", r=4))
            q_mms(4, 8)
            if B > 8:
                q_dma(8, min(8, B - 8))
            for c4 in range(4):
                wsb0[1][c4] = w_block(m2_t0, 4 + c4, c4, 1)
            q_mms(8, 16)
            pos = 16
            while pos < B:
                nb = min(8, B - pos)
                q_dma(pos, nb)
                q_mms(pos, pos + nb)
                pos += nb
            nc.sync.dma_start(qt_t[:, 4, :], q4_d[:])
            nc.sync.dma_start(bwe_t[:], bwe_d[:])
            for fb in range(4):
                if fb % 2 == 0:
                    nc.scalar.activation(qt_t[:, fb, :], qps[fb][:],
                                         IDENT, bias=0.0, scale=1.0)
                else:
                    nc.vector.tensor_copy(qt_t[:, fb, :], qps[fb][:])

            # Z for cq0 using the stored wsb tiles
            for g in range(2):
                osb = oop.tile([128, 4, 512], F32, tag="osb", name=f"osb0{g}")
                for c4 in range(4):
                    z_block(m2_t0, wsb0[g][c4], osb, g * 4 + c4, c4, g)
                    if c4 % 2 == 1:
                        base = g * 512 + (c4 - 1) * 128
                        nc.sync.dma_start(
                            out_d[base:base + 256, :]
                            .rearrange("(b p) j -> p b j", p=128),
                            osb[:, c4 - 1:c4 + 1, :])

            # remaining c-quarters: W+Z interleaved per 128-c block
            for cq in range(1, 4):
                m2_t = m2p.tile([128, 2, 4, 512], BF16, tag="m2_t",
                                name=f"m2_t{cq}")
                for g in range(2):
                    nc.sync.dma_start(
                        m2_t[:, g],
                        m2_d[:, cq * 4096 + g * 2048:cq * 4096 + (g + 1) * 2048]
                        .rearrange("p (r c) -> p r c", r=4))
                for g in range(2):
                    osb = oop.tile([128, 4, 512], F32, tag="osb",
                                   name=f"osb{cq}{g}")
                    for c4 in range(4):
                        cb = cq * 8 + g * 4 + c4
                        wsb = w_block(m2_t, cb, c4, g)
                        z_block(m2_t, wsb, osb, cb, c4, g)
                        if c4 % 2 == 1:
                            base = cq * 1024 + g * 512 + (c4 - 1) * 128
                            nc.sync.dma_start(
                                out_d[base:base + 256, :]
                                .rearrange("(b p) j -> p b j", p=128),
                                osb[:, c4 - 1:c4 + 1, :])
    nc.finalize()
    return nc


def _build_neff1():
    """Non-sym fallback.  Per core: wfT[:, J_m] = M2.T @ nf[J_m].T + bw."""
    nc = bacc.Bacc()
    m2_d = nc.dram_tensor("m2", [128, 16 * 1024], BF16, kind="ExternalInput")
    nfT_d = nc.dram_tensor("nfT", [128, 4 * JB], BF16, kind="ExternalInput")
    bw_d = nc.dram_tensor("bw", [128, 32], F32, kind="ExternalInput")
    wfT_d = nc.dram_tensor("wft_rows", [C, JB], BF16, kind="ExternalOutput")

    NRB = 4
    NCQ = 4

    with tile.TileContext(nc) as tc:
        with tc.tile_pool(name="const", bufs=1) as constp, \
             tc.tile_pool(name="m2p", bufs=2) as m2p, \
             tc.tile_pool(name="ps1", bufs=8, space=bass.MemorySpace.PSUM) as psp, \
             tc.tile_pool(name="io1", bufs=4) as iop:
            nfT_t = constp.tile([128, NRB, JB], BF16)
            nc.sync.dma_start(
                nfT_t[:], nfT_d[:].rearrange("p (r j) -> p r j", r=NRB))
            bw_t = constp.tile([128, 32], F32)
            nc.sync.dma_start(bw_t[:], bw_d[:])

            for cq in range(NCQ):
                m2_t = m2p.tile([128, 2, NRB, 512], BF16, tag="m2_t")
                for g in range(2):
                    nc.sync.dma_start(
                        m2_t[:, g, :, :],
                        m2_d[:, cq * 4096 + g * 2048:cq * 4096 + (g + 1) * 2048]
                        .rearrange("p (r c) -> p r c", r=NRB))
                for g in range(2):
                    o_sb = iop.tile([128, 4, 512], BF16, tag="o_sb")
                    for c4 in range(4):
                        cb = cq * 8 + g * 4 + c4
                        pw = psp.tile([128, 512], F32, tag="pw")
                        for rb in range(NRB):
                            nc.tensor.matmul(
                                pw[:],
                                m2_t[:, g, rb, c4 * 128:(c4 + 1) * 128],
                                nfT_t[:, rb, :],
                                start=(rb == 0), stop=(rb == NRB - 1))
                        if c4 % 2 == 0:
                            nc.scalar.activation(
                                o_sb[:, c4, :], pw[:], IDENT,
                                bias=bw_t[:, cb:cb + 1], scale=1.0)
                        else:
                            nc.vector.tensor_scalar_add(
                                o_sb[:, c4, :], pw[:], bw_t[:, cb:cb + 1])
                    nc.sync.dma_start(
                        wfT_d[cq * 1024 + g * 512:cq * 1024 + (g + 1) * 512, :]
                        .rearrange("(b p) j -> p b j", p=128), o_sb[:])
    nc.finalize()
    return nc


def _build_neff2():
    """Non-sym fallback.  PT cols J_m via fp8 DoubleRow, then out[:, J_m]."""
    nc = bacc.Bacc()
    ap_d = nc.dram_tensor("ap", [128, 8 * 16 * 2 * 512], FP8, kind="ExternalInput")
    aot_d = nc.dram_tensor("aot", [128, 16 * 2 * 512], FP8, kind="ExternalInput")
    sp_d = nc.dram_tensor("sp", [128, 8 * 4 * 512], BF16, kind="ExternalInput")
    wfp_d = nc.dram_tensor("wfp", [128, 8 * 32 * 512], BF16, kind="ExternalInput")
    wtp_d = nc.dram_tensor("wtp", [128, 8 * 4 * 512], F32, kind="ExternalInput")
    out_d = nc.dram_tensor("outc", [C, JB], F32, kind="ExternalOutput")

    NIS = 8
    NKBB = 16
    NCS = 8

    with tile.TileContext(nc) as tc:
        with tc.tile_pool(name="const", bufs=1) as constp, \
             tc.tile_pool(name="ga", bufs=2) as gap, \
             tc.tile_pool(name="gs", bufs=2) as gsp, \
             tc.tile_pool(name="wfpool", bufs=2) as wfpool, \
             tc.tile_pool(name="wtpool", bufs=2) as wtpool, \
             tc.tile_pool(name="oo", bufs=2) as oop:
            aot_t = constp.tile([128, NKBB, 2, 512], FP8)
            pt_t = constp.tile([128, 32, 512], BF16)

            with tc.tile_pool(name="psg", bufs=8, space=bass.MemorySpace.PSUM) as psgp:
              for isup in range(NIS):
                a_t = gap.tile([128, NKBB, 2, 512], FP8, tag="a_t")
                if isup == 0:
                    for q in range(4):
                        nc.sync.dma_start(
                            aot_t[:, q * 4:(q + 1) * 4, :, :],
                            aot_d[:, q * 4096:(q + 1) * 4096]
                            .rearrange("p (k h j) -> p k h j", k=4, h=2))
                        nc.sync.dma_start(
                            a_t[:, q * 4:(q + 1) * 4, :, :],
                            ap_d[:, q * 4096:(q + 1) * 4096]
                            .rearrange("p (k h i) -> p k h i", k=4, h=2))
                else:
                    for hf in range(2):
                        nc.sync.dma_start(
                            a_t[:, hf * 8:(hf + 1) * 8, :, :],
                            ap_d[:, isup * 16384 + hf * 8192:
                                 isup * 16384 + (hf + 1) * 8192]
                            .rearrange("p (k h i) -> p k h i", k=8, h=2))
                s_t = gsp.tile([128, 4, 512], BF16, tag="s_t")
                nc.sync.dma_start(
                    s_t[:],
                    sp_d[:, isup * 2048:(isup + 1) * 2048]
                    .rearrange("p (b j) -> p b j", b=4))
                psg = [psgp.tile([128, 512], F32, tag="psg", name=f"psg{i}")
                       for i in range(4)]
                for kbb in range(NKBB):
                    for ib4 in range(4):
                        nc.tensor.matmul(
                            psg[ib4][:],
                            a_t[:, kbb, :, ib4 * 128:(ib4 + 1) * 128],
                            aot_t[:, kbb, :, :],
                            start=(kbb == 0), stop=(kbb == NKBB - 1),
                            perf_mode=DR)
                for ib4 in range(4):
                    nc.vector.tensor_mul(
                        pt_t[:, isup * 4 + ib4, :], psg[ib4][:], s_t[:, ib4, :])

            with tc.tile_pool(name="pso", bufs=8, space=bass.MemorySpace.PSUM) as psop:
              for csup in range(NCS):
                wf_t = wfpool.tile([128, 32, 512], BF16, tag="wf_t")
                nc.sync.dma_start(
                    wf_t[:],
                    wfp_d[:, csup * 16384:(csup + 1) * 16384]
                    .rearrange("p (b c) -> p b c", b=32))
                wt_t = wtpool.tile([128, 4, 512], F32, tag="wt_t")
                nc.sync.dma_start(
                    wt_t[:],
                    wtp_d[:, csup * 2048:(csup + 1) * 2048]
                    .rearrange("p (b j) -> p b j", b=4))
                pso = [psop.tile([128, 512], F32, tag="pso", name=f"pso{i}")
                       for i in range(4)]
                for ib in range(32):
                    for cb in range(4):
                        nc.tensor.matmul(
                            pso[cb][:],
                            wf_t[:, ib, cb * 128:(cb + 1) * 128],
                            pt_t[:, ib, :],
                            start=(ib == 0), stop=(ib == 31))
                for half in range(2):
                    o_sb = oop.tile([128, 2, 512], F32, tag="o_sb")
                    for c2 in range(2):
                        cb = half * 2 + c2
                        nc.vector.tensor_mul(
                            o_sb[:, c2, :], pso[cb][:], wt_t[:, cb, :])
                    nc.sync.dma_start(
                        out_d[csup * 512 + half * 256:
                              csup * 512 + (half + 1) * 256, :]
                        .rearrange("(b p) j -> p b j", p=128), o_sb[:])
    nc.finalize()
    return nc


# ---- host-side packing helpers ----

def _pack_m2_bw(lw, lb, W):
    M2 = (lw.T @ W).astype(np.float32)          # [F_RAW, C]
    bw = (lb.astype(np.float64) @ W.astype(np.float64)).astype(np.float32)
    m2p = np.ascontiguousarray(
        M2.reshape(4, 128, 4, 2, 512).transpose(1, 2, 3, 0, 4).reshape(128, -1)
        .astype(BF))
    return m2p, bw


def _pack_nfT(nf, m):
    nfT = nf[m * JB:(m + 1) * JB, :].T  # [F_RAW, JB]
    return np.ascontiguousarray(
        nfT.reshape(4, 128, JB).transpose(1, 0, 2).reshape(128, -1).astype(BF))


def _pack_a_fp8(A):
    # (p, isup, kbb, h, i) with k = kbb*256 + h*128 + p
    a8 = A.astype(F8)
    return np.ascontiguousarray(
        a8.reshape(16, 2, 128, 8, 512).transpose(2, 3, 0, 1, 4).reshape(128, -1))


def _pack_cols_kh(X, dtype):
    # X [N, JB] -> (p, kbb, h, j) with k = kbb*256 + h*128 + p
    return np.ascontiguousarray(
        X.astype(dtype).reshape(16, 2, 128, JB).transpose(2, 0, 1, 3).reshape(128, -1))


def _pack_rows_sup(X, dtype, nsup, nb):
    # X [N, JB] -> (p, sup, b, j) with row = sup*512 + b*128 + p
    return np.ascontiguousarray(
        X.astype(dtype).reshape(nsup, nb, 128, -1).transpose(2, 0, 1, 3).reshape(128, -1))


def _pack_rows128(X):
    # X [nb*128, F] -> (p, b, f) with row = b*128 + p
    nb = X.shape[0] // 128
    return np.ascontiguousarray(
        X.reshape(nb, 128, -1).transpose(1, 0, 2).reshape(128, -1).astype(BF))


def _pack_c32(v):
    # v [4096] -> [128, 32] with c = cb*128 + p
    return np.ascontiguousarray(v.reshape(32, 128).T)


_NCS = {}


def _get_nc(name):
    if name not in _NCS:
        if name.startswith("no"):
            _NCS[name] = _build_neffo(int(name[2:]))
        elif name.startswith("ng"):
            kb0, kbp = name[2:].split("_")
            _NCS[name] = _build_neffg(int(kb0), int(kbp))
        else:
            _NCS[name] = {"n1": _build_neff1, "n2": _build_neff2}[name]()
    return _NCS[name]


def _ensure_trace_hook():
    """Best-effort NTFF profiling shim (test harness only; grading runs
    without tracing)."""
    try:
        from antenv.axon_hooks import get_axon_ntff_profile_hook
        return get_axon_ntff_profile_hook() is not None
    except ImportError:
        pass
    try:
        import types
        if "/root/.axon_site" not in sys.path:
            sys.path.insert(0, "/root/.axon_site")
        from trn_agent_boot.trn_boot import _ntff_profile_via_ctypes
        hook = _ntff_profile_via_ctypes("/opt/axon/libaxon_pjrt.so")
        if hook is None:
            return False
        import antenv
        mod = types.ModuleType("antenv.axon_hooks")
        mod.get_axon_ntff_profile_hook = lambda: hook
        mod.set_axon_ntff_profile_hook = lambda h: None
        sys.modules["antenv.axon_hooks"] = mod
        antenv.axon_hooks = mod
        from concourse import bass_utils as _bu
        _bu.upload_artifacts = lambda tmpdir: ""
        return True
    except Exception:
        return False


def _run(nc, in_maps, cores, trace, tag):
    if trace:
        try:
            r = run_bass_kernel_spmd(nc, in_maps, cores, trace=True)
            LAST_EXEC[tag] = r.exec_time_ns
            LAST_RESULTS[tag] = r
            return r
        except Exception as e:
            print(f"trace run failed ({e!r}); retrying without trace")
    return run_bass_kernel_spmd(nc, in_maps, cores)


def kernel(node_features, adjacency_matrix, mask_father, neighbor_count,
           mask_hadamard, linear_w, linear_b, weight):
    trace = bool(int(os.environ.get("BASS_KERNEL_TRACE", "0"))) and _ensure_trace_hook()
    cores = list(range(M))

    nf = np.ascontiguousarray(np.asarray(node_features, dtype=np.float32))
    A = np.ascontiguousarray(np.asarray(adjacency_matrix, dtype=np.float32))
    Ao = np.ascontiguousarray(np.asarray(mask_father, dtype=np.float32)[:, 0, :])
    S = np.ascontiguousarray(np.asarray(mask_hadamard, dtype=np.float32)[:, 0, :])
    ncnt = np.asarray(neighbor_count, dtype=np.float32)
    lw = np.asarray(linear_w, dtype=np.float32)
    lb = np.asarray(linear_b, dtype=np.float32)
    W = np.ascontiguousarray(np.asarray(weight, dtype=np.float32))

    inv2 = (1.0 / np.square(ncnt.astype(np.float64)))[:, 0].astype(np.float32)
    m2p, bw = _pack_m2_bw(lw, lb, W)

    # mask_father == adjacency makes G = A^T A symmetric; the sym path
    # computes only 5/8 of G per core in launch 1 and mirrors on the host.
    sym = np.array_equal(Ao, A)

    if sym:
        # ---- launch 1: G blocks only ----
        a8 = A.astype(F8)
        # per-core contraction pruning: drop k-rows with A[k, C_m] == 0
        # (zero moving row => zero contribution to every block).  Shared-
        # moving mode (KBp=0): one permutation for all slots, the t0 tile
        # doubles as every moving operand — least DMA, which wins when the
        # device power-throttle squeezes bandwidth.
        pi0s = [np.nonzero(A[:, m * 512:(m + 1) * 512].any(axis=1))[0]
                for m in range(M)]
        KB0 = max(1, max((len(p) + 255) // 256 for p in pi0s))

        def _slotpack(X):
            kbx = X.shape[0] // 256
            return X.reshape(kbx, 2, 128, 512).transpose(2, 0, 1, 3) \
                    .reshape(128, -1)

        in1 = []
        for m in range(M):
            sl = slice(m * 512, (m + 1) * 512)
            pi = pi0s[m]
            stats = []
            a0 = np.zeros((KB0 * 256, 512), dtype=F8)
            a0[:len(pi)] = a8[pi, sl]
            stats.append(a0)
            for t in range(1, 5):
                bi = (m + t) % 8
                cols = np.arange(bi * 512, (bi + 1) * 512)
                if t == 4 and m >= 4:
                    cols = np.concatenate([cols[256:], cols[:256]])
                sb = np.zeros((KB0 * 256, 512), dtype=F8)
                sb[:len(pi)] = a8[np.ix_(pi, cols)]
                stats.append(sb)
            in1.append({"apm": np.ascontiguousarray(
                np.concatenate([_slotpack(s) for s in stats], axis=1))})
        r1 = _run(_get_nc(f"ng{KB0}_0"), in1, cores, trace, "neff1")

        # assemble full G from the pieces + symmetry, mask with S
        Gf = np.empty((N, N), dtype=np.float32)
        g1s = [r1.results[m]["g1"] for m in range(M)]
        for m in range(M):
            # diag block from the symmetric-recursion pieces
            D = np.empty((512, 512), dtype=np.float32)
            D[:, 0:256] = g1s[m][0:512, 0:256]
            D[0:256, 256:512] = g1s[m][256:512, 0:256].T
            D[256:512, 256:384] = g1s[m][0:256, 256:384]
            D[256:384, 384:512] = g1s[m][128:256, 256:384].T
            D[384:512, 384:512] = g1s[m][0:128, 384:512]
            Gf[m * 512:(m + 1) * 512, m * 512:(m + 1) * 512] = D
            for t in range(1, 4):
                bi = (m + t) % 8
                Gf[bi * 512:(bi + 1) * 512, m * 512:(m + 1) * 512] = \
                    g1s[m][t * 512:(t + 1) * 512, :]
        for m in range(M):
            for d in (5, 6, 7):
                bi = (m + d) % 8
                tp = (m - bi) % 8
                Gf[bi * 512:(bi + 1) * 512, m * 512:(m + 1) * 512] = \
                    g1s[bi][tp * 512:(tp + 1) * 512, :].T
        # pair blocks {p, p+4}: four 256x256 quadrants split across the pair
        for p in range(4):
            q = p + 4
            B = np.empty((512, 512), dtype=np.float32)
            B[0:256, 0:256] = g1s[p][2048:2304, 0:256]
            B[256:512, 256:512] = g1s[p][2304:2560, 0:256]
            B[0:256, 256:512] = g1s[q][2048:2304, 0:256].T
            B[256:512, 0:256] = g1s[q][2304:2560, 0:256].T
            Gf[q * 512:(q + 1) * 512, p * 512:(p + 1) * 512] = B
            Gf[p * 512:(p + 1) * 512, q * 512:(q + 1) * 512] = B.T
        pt = Gf * S  # PT[i, j] (G symmetric)

        # ---- launch 2: factored O phase ----
        # drop PT rows that are all-zero for each core's columns (exact);
        # B = max block count across cores keeps the NEFF uniform (SPMD)
        bwe = np.zeros((128, C), dtype=BF)
        bwe[0, :] = bw.astype(BF)
        bvip = np.ascontiguousarray(np.concatenate(
            [_pack_c32((bw.astype(np.float64) * inv2).astype(np.float32)),
             _pack_c32(inv2)], axis=1))
        parts = []
        for m in range(M):
            sl = slice(m * JB, (m + 1) * JB)
            ptm = np.ascontiguousarray(pt[:, sl])
            rs = ptm.sum(axis=0)
            nz = np.nonzero(ptm.any(axis=1))[0]
            parts.append((ptm, rs, nz))
        B = max(1, max((len(nz) + 127) // 128 for _, _, nz in parts))
        in2 = []
        for m, (ptm, rs, nz) in enumerate(parts):
            pt_sel = np.zeros((B * 128, JB), dtype=np.float32)
            pt_sel[:len(nz)] = ptm[nz]
            nf_sel = np.zeros((B * 128, F_RAW), dtype=np.float32)
            nf_sel[:len(nz)] = nf[nz]
            q4 = np.zeros((128, 512), dtype=BF)
            q4[0, :] = rs.astype(BF)
            in2.append({
                "ptp": _pack_rows128(pt_sel),
                "nfp": _pack_rows128(nf_sel),
                "m2": m2p,
                "nfT": _pack_nfT(nf, m),
                "bwe": bwe,
                "q4": q4,
                "bvi": bvip,
            })
        r2 = _run(_get_nc(f"no{B}"), in2, cores, trace, "neff2")
    else:
        # ---- fallback: original two-launch path ----
        bwp = _pack_c32(bw)
        in1 = [{"m2": m2p, "nfT": _pack_nfT(nf, m), "bw": bwp}
               for m in range(M)]
        r1 = _run(_get_nc("n1"), in1, cores, trace, "neff1")
        wfT = np.concatenate([r1.results[m]["wft_rows"] for m in range(M)],
                             axis=1)
        wfb = np.ascontiguousarray(wfT.T)  # [N, C] bf16
        wfT32 = wfT.astype(np.float32)
        wfp = np.ascontiguousarray(
            wfb.reshape(32, 128, 8, 512).transpose(1, 2, 0, 3).reshape(128, -1))
        a_pack = _pack_a_fp8(A)
        in2 = []
        for m in range(M):
            sl = slice(m * JB, (m + 1) * JB)
            wt = wfT32[:, sl] * inv2[:, None]
            in2.append({
                "ap": a_pack,
                "aot": _pack_cols_kh(np.ascontiguousarray(Ao[:, sl]), F8),
                "sp": _pack_rows_sup(np.ascontiguousarray(S[:, sl]), BF, 8, 4),
                "wfp": wfp,
                "wtp": _pack_rows_sup(wt.astype(np.float32), np.float32, 8, 4),
            })
        r2 = _run(_get_nc("n2"), in2, cores, trace, "neff2")

    out = np.empty((C, N), dtype=np.float32)
    for m in range(M):
        out[:, m * JB:(m + 1) * JB] = r2.results[m]["outc"]
    return out


# revision 61
# speedup vs baseline: 1.1354x; 1.0189x over previous
"""Distributed Bass kernel for nn_Interaction_GraphConvolution.

Math (reference):
    x  = node_features @ linear_w.T + linear_b          [N, IN_F]
    wf = x @ weight                                     [N, C]
    G  = mask_father[:,0,:].T @ adjacency               [N, N]
    P  = G * mask_hadamard[:,0,:].T                     [N, N]
    out[c, j] = wf[j,c] * (P @ wf)[j,c] / neighbor_count[c]^2

Key factorization (sym fast path): wf = nf @ M2 + 1*bw with
M2 = lw.T @ W (host, weights only), so
    P @ wf = (P @ nf) @ M2 + (P @ 1) x bw
turning the [N,N]@[N,C] GEMM (17.2 GF/core) into two skinny GEMMs
through the rank-512 bottleneck (2.15 GF each) plus a rank-1 term.

Sharding: output columns j (node dim) split across 8 cores, 512 each.
Two SPMD launches:
  NEFF-G: core m computes its diagonal G block via symmetric recursion,
          3 full off-diagonal row-blocks, and two 256x256 quadrants of
          the shared {m, m+4} pair block of symmetric G = A^T A
          (fp8 DoubleRow, exact on 0/1).  k-rows of A that are zero over
          the core's columns are dropped host-side (exact).
  NEFF-O: core m computes Q^T = nf^T @ PT[:,J_m] (PT rows that are
          all-zero dropped host-side), then Z^T = M2^T @ Q^T + bw x rs
          (rs = PT colsums, 5th k-block), wfT[:,J_m] on-core, and
          out = Z^T * wfT * inv2.

All DRAM operands are host-packed so each DMA moves a multi-KB
contiguous line per partition; DMA issue order puts each launch's
smallest PE-unblocking input first.
"""

import os
import sys

sys.path.insert(0, "/opt/trn_rl_repo")

import numpy as np
import ml_dtypes

from concourse import bass, bacc, mybir, tile
from concourse.bass_utils import run_bass_kernel_spmd

F32 = mybir.dt.float32
BF16 = mybir.dt.bfloat16
FP8 = mybir.dt.float8e4
DR = mybir.MatmulPerfMode.DoubleRow
IDENT = mybir.ActivationFunctionType.Identity

BF = ml_dtypes.bfloat16
F8 = ml_dtypes.float8_e4m3fn

N = 4096       # nodes (== out channels C)
F_RAW = 512    # raw feature dim
IN_F = 1024    # hidden dim
C = 4096       # out channels
M = 8          # cores
JB = N // M    # 512 output columns per core

LAST_EXEC = {}
LAST_RESULTS = {}


def _build_neffg(KB0=16, KBp=16):
    """G half via symmetry.  Core m computes, for its columns C_m:
      t=0 diag block D = G[C_m, C_m] via symmetric recursion: D[:, 0:256],
          then D[256:512, 256:384] and D[384:512, 384:512] (the rest is
          mirrored on the host) — all operands are slices of one A tile.
      t=1..3 full row-blocks (m+t)%8.
      t=4 two 256x256 quadrants of the shared {m, m+4} pair block (for
          m >= 4 the stat column halves are swapped host-side so the pair
          covers all four quadrants with no duplication).
    Contraction pruning: k-rows of A that are zero over the relevant
    column sets are dropped host-side (exact).  KB0 = 256-row k-blocks for
    t=0 (A[k, C_m] != 0); KBp = k-blocks for t=1..4 (nonzero on BOTH the
    stat and moving column sets) — each t has its own permuted stat AND
    moving copy, aligned row-by-row.

    KBp == 0 selects the shared-moving mode: t=1..4 stats are packed with
    the t0 permutation (KB0 blocks) and the moving operand is the t0 tile
    itself — 6 MB less DMA, a few more matmuls; the right trade when the
    device power-throttle squeezes DMA bandwidth.

    apm [128, (KB0+4*KBx)*1024] fp8 : slot 0 = t0 operand (stat==moving);
                                      slots 1..4 = stats, (p, kbb, h, i)
    aom [128, 4*KBp*1024] fp8       : movings for t=1..4 (absent if KBp=0)
    out g1 [5*512, JB] bf16 : t0 pieces at [0:512, 0:256], [0:256,
        256:384], [0:128, 384:512]; t1-3 full rows t*512; t4 cols 0:256
    """
    KBx = KBp if KBp > 0 else KB0
    nc = bacc.Bacc()
    apm_d = nc.dram_tensor("apm", [128, (KB0 + 4 * KBx) * 1024], FP8,
                           kind="ExternalInput")
    if KBp > 0:
        aom_d = nc.dram_tensor("aom", [128, 4 * KBp * 1024], FP8,
                               kind="ExternalInput")
    g1_d = nc.dram_tensor("g1", [5 * 512, JB], BF16, kind="ExternalOutput")

    with tile.TileContext(nc) as tc:
        with tc.tile_pool(name="const", bufs=1) as constp, \
             tc.tile_pool(name="ga", bufs=3) as gap, \
             tc.tile_pool(name="gm", bufs=3) as gmp, \
             tc.tile_pool(name="io1", bufs=4) as iop, \
             tc.tile_pool(name="psg", bufs=8, space=bass.MemorySpace.PSUM) as psgp:
            aot_t = constp.tile([128, KB0, 2, 512], FP8)
            pos = 0
            while pos < KB0:
                nk = min(2, KB0 - pos)
                nc.sync.dma_start(
                    aot_t[:, pos:pos + nk, :, :],
                    apm_d[:, pos * 1024:(pos + nk) * 1024]
                    .rearrange("p (k h j) -> p k h j", k=nk, h=2))
                pos += nk

            # t0: P1 = D[:, 0:256], P2a = D[256:512, 256:384],
            #     P2b = D[384:512, 384:512]
            psg = [psgp.tile([128, 512], F32, tag="psg", name=f"psg0_{i}")
                   for i in range(4)]
            ps2 = [psgp.tile([128, 512], F32, tag="psg", name=f"psg0b_{i}")
                   for i in range(3)]
            for kbb in range(KB0):
                st, sp = (kbb == 0), (kbb == KB0 - 1)
                for ib4 in range(4):
                    nc.tensor.matmul(
                        psg[ib4][:, 0:256],
                        aot_t[:, kbb, :, ib4 * 128:(ib4 + 1) * 128],
                        aot_t[:, kbb, :, 0:256],
                        start=st, stop=sp, perf_mode=DR)
                for i2 in range(2):
                    nc.tensor.matmul(
                        ps2[i2][:, 0:128],
                        aot_t[:, kbb, :, (2 + i2) * 128:(3 + i2) * 128],
                        aot_t[:, kbb, :, 256:384],
                        start=st, stop=sp, perf_mode=DR)
                nc.tensor.matmul(
                    ps2[2][:, 0:128],
                    aot_t[:, kbb, :, 384:512],
                    aot_t[:, kbb, :, 384:512],
                    start=st, stop=sp, perf_mode=DR)
            og1 = iop.tile([128, 4, 256], BF16, tag="og256")
            for ib4 in range(4):
                if ib4 % 2 == 0:
                    nc.scalar.activation(og1[:, ib4, :], psg[ib4][:, 0:256],
                                         IDENT, bias=0.0, scale=1.0)
                else:
                    nc.vector.tensor_copy(og1[:, ib4, :], psg[ib4][:, 0:256])
            nc.sync.dma_start(
                g1_d[0:512, 0:256].rearrange("(b p) j -> p b j", p=128),
                og1[:])
            og2 = iop.tile([128, 3, 128], BF16, tag="og128")
            for i2 in range(3):
                if i2 % 2 == 0:
                    nc.scalar.activation(og2[:, i2, :], ps2[i2][:, 0:128],
                                         IDENT, bias=0.0, scale=1.0)
                else:
                    nc.vector.tensor_copy(og2[:, i2, :], ps2[i2][:, 0:128])
            nc.sync.dma_start(
                g1_d[0:256, 256:384].rearrange("(b p) j -> p b j", p=128),
                og2[:, 0:2, :])
            nc.sync.dma_start(
                g1_d[0:128, 384:512].rearrange("(b p) j -> p b j", p=128),
                og2[:, 2:3, :])

            # t1..4: stat from apm slot t; moving from aom slot t-1, or the
            # t0 tile in shared-moving mode
            for t in range(1, 5):
                a_t = gap.tile([128, KBx, 2, 512], FP8, tag="a_t")
                sbase = (KB0 + (t - 1) * KBx) * 1024
                mbase = (t - 1) * KBx * 1024
                ao_t = aot_t if KBp == 0 else \
                    gmp.tile([128, KBx, 2, 512], FP8, tag="ao_t")
                pos = 0
                while pos < KBx:
                    nk = min(8, KBx - pos)
                    nc.sync.dma_start(
                        a_t[:, pos:pos + nk, :, :],
                        apm_d[:, sbase + pos * 1024:sbase + (pos + nk) * 1024]
                        .rearrange("p (k h i) -> p k h i", k=nk, h=2))
                    if KBp > 0:
                        nc.sync.dma_start(
                            ao_t[:, pos:pos + nk, :, :],
                            aom_d[:, mbase + pos * 1024:
                                  mbase + (pos + nk) * 1024]
                            .rearrange("p (k h i) -> p k h i", k=nk, h=2))
                    pos += nk
                psgt = [psgp.tile([128, 512], F32, tag="psg",
                                  name=f"psg{t}_{i}") for i in range(4)]
                half = 512 if t < 4 else 256
                for kbb in range(KBx):
                    for ib4 in range(4):
                        jlo = 0 if (t < 4 or ib4 < 2) else 256
                        nc.tensor.matmul(
                            psgt[ib4][:, 0:half],
                            a_t[:, kbb, :, ib4 * 128:(ib4 + 1) * 128],
                            ao_t[:, kbb, :, jlo:jlo + half],
                            start=(kbb == 0), stop=(kbb == KBx - 1),
                            perf_mode=DR)
                og = iop.tile([128, 4, half], BF16, tag=f"ogt{half}")
                for ib4 in range(4):
                    if ib4 % 2 == 0:
                        nc.scalar.activation(og[:, ib4, :],
                                             psgt[ib4][:, 0:half],
                                             IDENT, bias=0.0, scale=1.0)
                    else:
                        nc.vector.tensor_copy(og[:, ib4, :],
                                              psgt[ib4][:, 0:half])
                nc.sync.dma_start(
                    g1_d[t * 512:(t + 1) * 512, 0:half]
                    .rearrange("(b p) j -> p b j", p=128), og[:])
    nc.finalize()
    return nc


def _build_neffo(B=32):
    """Factored O phase.  B = number of 128-row i-blocks kept after the
    host drops PT rows that are all-zero for this core's columns (the
    same row permutation is applied to nf, so the contraction is exact).

    ptp [128, B*512] bf16  : PT[:, J_m] packed (p, ib, j), i = ib*128+p
    nfp [128, B*512] bf16  : nf packed (p, ib, f), i = ib*128+p
    m2  [128, 16*1024] bf16: M2 packed (p, cq, g, rb, cw), f = rb*128+p,
                             c = cq*1024 + g*512 + cw
    nfT [128, 4*512] bf16  : nf[J_m].T packed (p, rb, j), f = rb*128+p
    bwe [128, 4096] bf16   : row 0 = bw (stat for the rank-1 term; doing
                             the rank-1 on scalar/DVE/GpSimd instead was
                             measured SLOWER — GpSimd tensor ops are
                             1270 ns per [128,512] and the cross-engine
                             chain beats the 32 matmuls it saves)
    q4  [128, 512] bf16    : row 0 = rs = colsums of PT[:, J_m]
    bvi [128, 64] f32      : cols 0:32 = bw*inv2, cols 32:64 = inv2,
                             packed c = cb*128+p
    out outc [C, JB] f32
    """
    nc = bacc.Bacc()
    ptp_d = nc.dram_tensor("ptp", [128, B * 512], BF16, kind="ExternalInput")
    nfp_d = nc.dram_tensor("nfp", [128, B * 512], BF16, kind="ExternalInput")
    m2_d = nc.dram_tensor("m2", [128, 16 * 1024], BF16, kind="ExternalInput")
    nfT_d = nc.dram_tensor("nfT", [128, 4 * JB], BF16, kind="ExternalInput")
    bwe_d = nc.dram_tensor("bwe", [128, C], BF16, kind="ExternalInput")
    q4_d = nc.dram_tensor("q4", [128, 512], BF16, kind="ExternalInput")
    bvi_d = nc.dram_tensor("bvi", [128, 64], F32, kind="ExternalInput")
    out_d = nc.dram_tensor("outc", [C, JB], F32, kind="ExternalOutput")

    with tile.TileContext(nc) as tc:
        with tc.tile_pool(name="const", bufs=1) as constp, \
             tc.tile_pool(name="m2p", bufs=2) as m2p, \
             tc.tile_pool(name="wz", bufs=4) as wzp, \
             tc.tile_pool(name="oo", bufs=4) as oop, \
             tc.tile_pool(name="ps2", bufs=8, space=bass.MemorySpace.PSUM) as psp:
            ptp_t = constp.tile([128, B, 512], BF16)
            nfp_t = constp.tile([128, B, 512], BF16)
            qt_t = constp.tile([128, 5, 512], BF16)
            nfT_t = constp.tile([128, 4, JB], BF16)
            bwe_t = constp.tile([128, C], BF16)
            bvi_t = constp.tile([128, 64], F32)
            bwi_t = bvi_t[:, 0:32]
            inv_t = bvi_t[:, 32:64]

            # critical-path DMA order: Q's first 4-block chunk (1 MB) is the
            # smallest dependency that lets the PE start; the W inputs and
            # the rest of the Q stream land behind it.
            nc.sync.dma_start(bvi_t, bvi_d[:])
            m2_t0 = m2p.tile([128, 2, 4, 512], BF16, tag="m2_t", name="m2_t0")

            def w_block(m2_t, cb, c4, g):
                pwf = psp.tile([128, 512], F32, tag="pwf", bufs=2,
                               name=f"pwf{cb}")
                for rb in range(4):
                    nc.tensor.matmul(
                        pwf[:],
                        m2_t[:, g, rb, c4 * 128:(c4 + 1) * 128],
                        nfT_t[:, rb, :],
                        start=(rb == 0), stop=(rb == 3))
                wsb = wzp.tile([128, 512], F32, tag="wsb", bufs=8,
                               name=f"wsb{cb}")
                nc.scalar.activation(
                    wsb[:], pwf[:], IDENT,
                    bias=bwi_t[:, cb:cb + 1], scale=inv_t[:, cb:cb + 1])
                return wsb

            def z_block(m2_t, wsb, osb, cb, c4, g):
                pz = psp.tile([128, 512], F32, tag="pz", bufs=2,
                              name=f"pz{cb}")
                for rb in range(4):
                    nc.tensor.matmul(
                        pz[:],
                        m2_t[:, g, rb, c4 * 128:(c4 + 1) * 128],
                        qt_t[:, rb, :],
                        start=(rb == 0), stop=False)
                nc.tensor.matmul(
                    pz[:],
                    bwe_t[:, cb * 128:(cb + 1) * 128],
                    qt_t[:, 4, :],
                    start=False, stop=True)
                nc.vector.tensor_mul(osb[:, c4, :], pz[:], wsb[:])

            def q_dma(pos, nb):
                nc.sync.dma_start(
                    nfp_t[:, pos:pos + nb, :],
                    nfp_d[:, pos * 512:(pos + nb) * 512]
                    .rearrange("p (b f) -> p b f", b=nb))
                nc.sync.dma_start(
                    ptp_t[:, pos:pos + nb, :],
                    ptp_d[:, pos * 512:(pos + nb) * 512]
                    .rearrange("p (b j) -> p b j", b=nb))

            qps = [psp.tile([128, 512], F32, tag="qps", name=f"qps{i}", bufs=4)
                   for i in range(4)]

            def q_mms(lo, hi):
                # Q^T[f, j] += sum_i nf[i, f] * PT[i, j], i-blocks [lo, hi)
                for ib in range(lo, min(hi, B)):
                    for fb in range(4):
                        nc.tensor.matmul(
                            qps[fb][:],
                            nfp_t[:, ib, fb * 128:(fb + 1) * 128],
                            ptp_t[:, ib, :],
                            start=(ib == 0), stop=(ib == B - 1))

            # interleave: Q chunks first (smallest PE-unblocking data), W g0
            # and g1 slotted in as their inputs land.  The W accumulation
            # groups use separate PSUM banks, so they nest freely inside
            # Q's open accumulation chain.
            wsb0 = [[None] * 4 for _ in range(2)]
            # issue the four critical head transfers from four different
            # engine queues so their ~0.65us issue costs run concurrently
            # (all engines are otherwise idle at the head)
            n0 = min(2, B)
            nc.sync.dma_start(
                nfp_t[:, 0:n0, :],
                nfp_d[:, 0:n0 * 512].rearrange("p (b f) -> p b f", b=n0))
            nc.scalar.dma_start(
                ptp_t[:, 0:n0, :],
                ptp_d[:, 0:n0 * 512].rearrange("p (b j) -> p b j", b=n0))
            if B > 2:
                n1 = min(2, B - 2)
                nc.gpsimd.dma_start(
                    nfp_t[:, 2:2 + n1, :],
                    nfp_d[:, 1024:(2 + n1) * 512]
                    .rearrange("p (b f) -> p b f", b=n1))
                nc.sync.dma_start(
                    ptp_t[:, 2:2 + n1, :],
                    ptp_d[:, 1024:(2 + n1) * 512]
                    .rearrange("p (b j) -> p b j", b=n1))
            nc.scalar.dma_start(bvi_t, bvi_d[:])
            q_mms(0, 4)
            nc.sync.dma_start(nfT_t[:],
                              nfT_d[:].rearrange("p (r j) -> p r j", r=4))
            nc.sync.dma_start(
                m2_t0[:, 0],
                m2_d[:, 0:2048].rearrange("p (r c) -> p r c", r=4))
            if B > 4:
                q_dma(4, min(4, B - 4))
            for c4 in range(4):
                wsb0[0][c4] = w_block(m2_t0, c4, c4, 0)
            nc.sync.dma_start(
                m2_t0[:, 1],
                m2_d[:, 2048:4096].rearrange("p (r c) -> p r c", r=4))
            q_mms(4, 8)
            if B > 8:
                q_dma(8, min(8, B - 8))
            for c4 in range(4):
                wsb0[1][c4] = w_block(m2_t0, 4 + c4, c4, 1)
            q_mms(8, 16)
            pos = 16
            while pos < B:
                nb = min(8, B - pos)
                q_dma(pos, nb)
                q_mms(pos, pos + nb)
                pos += nb
            nc.sync.dma_start(qt_t[:, 4, :], q4_d[:])
            nc.sync.dma_start(bwe_t[:], bwe_d[:])
            for fb in range(4):
                if fb % 2 == 0:
                    nc.scalar.activation(qt_t[:, fb, :], qps[fb][:],
                                         IDENT, bias=0.0, scale=1.0)
                else:
                    nc.vector.tensor_copy(qt_t[:, fb, :], qps[fb][:])

            # Z for cq0 using the stored wsb tiles
            for g in range(2):
                osb = oop.tile([128, 4, 512], F32, tag="osb", name=f"osb0{g}")
                for c4 in range(4):
                    z_block(m2_t0, wsb0[g][c4], osb, g * 4 + c4, c4, g)
                    if c4 % 2 == 1:
                        base = g * 512 + (c4 - 1) * 128
                        nc.sync.dma_start(
                            out_d[base:base + 256, :]
                            .rearrange("(b p) j -> p b j", p=128),
                            osb[:, c4 - 1:c4 + 1, :])

            # remaining c-quarters: W+Z interleaved per 128-c block
            for cq in range(1, 4):
                m2_t = m2p.tile([128, 2, 4, 512], BF16, tag="m2_t",
                                name=f"m2_t{cq}")
                for g in range(2):
                    nc.sync.dma_start(
                        m2_t[:, g],
                        m2_d[:, cq * 4096 + g * 2048:cq * 4096 + (g + 1) * 2048]
                        .rearrange("p (r c) -> p r c", r=4))
                for g in range(2):
                    osb = oop.tile([128, 4, 512], F32, tag="osb",
                                   name=f"osb{cq}{g}")
                    for c4 in range(4):
                        cb = cq * 8 + g * 4 + c4
                        wsb = w_block(m2_t, cb, c4, g)
                        z_block(m2_t, wsb, osb, cb, c4, g)
                        if c4 % 2 == 1:
                            base = cq * 1024 + g * 512 + (c4 - 1) * 128
                            nc.sync.dma_start(
                                out_d[base:base + 256, :]
                                .rearrange("(b p) j -> p b j", p=128),
                                osb[:, c4 - 1:c4 + 1, :])
    nc.finalize()
    return nc


def _build_neff1():
    """Non-sym fallback.  Per core: wfT[:, J_m] = M2.T @ nf[J_m].T + bw."""
    nc = bacc.Bacc()
    m2_d = nc.dram_tensor("m2", [128, 16 * 1024], BF16, kind="ExternalInput")
    nfT_d = nc.dram_tensor("nfT", [128, 4 * JB], BF16, kind="ExternalInput")
    bw_d = nc.dram_tensor("bw", [128, 32], F32, kind="ExternalInput")
    wfT_d = nc.dram_tensor("wft_rows", [C, JB], BF16, kind="ExternalOutput")

    NRB = 4
    NCQ = 4

    with tile.TileContext(nc) as tc:
        with tc.tile_pool(name="const", bufs=1) as constp, \
             tc.tile_pool(name="m2p", bufs=2) as m2p, \
             tc.tile_pool(name="ps1", bufs=8, space=bass.MemorySpace.PSUM) as psp, \
             tc.tile_pool(name="io1", bufs=4) as iop:
            nfT_t = constp.tile([128, NRB, JB], BF16)
            nc.sync.dma_start(
                nfT_t[:], nfT_d[:].rearrange("p (r j) -> p r j", r=NRB))
            bw_t = constp.tile([128, 32], F32)
            nc.sync.dma_start(bw_t[:], bw_d[:])

            for cq in range(NCQ):
                m2_t = m2p.tile([128, 2, NRB, 512], BF16, tag="m2_t")
                for g in range(2):
                    nc.sync.dma_start(
                        m2_t[:, g, :, :],
                        m2_d[:, cq * 4096 + g * 2048:cq * 4096 + (g + 1) * 2048]
                        .rearrange("p (r c) -> p r c", r=NRB))
                for g in range(2):
                    o_sb = iop.tile([128, 4, 512], BF16, tag="o_sb")
                    for c4 in range(4):
                        cb = cq * 8 + g * 4 + c4
                        pw = psp.tile([128, 512], F32, tag="pw")
                        for rb in range(NRB):
                            nc.tensor.matmul(
                                pw[:],
                                m2_t[:, g, rb, c4 * 128:(c4 + 1) * 128],
                                nfT_t[:, rb, :],
                                start=(rb == 0), stop=(rb == NRB - 1))
                        if c4 % 2 == 0:
                            nc.scalar.activation(
                                o_sb[:, c4, :], pw[:], IDENT,
                                bias=bw_t[:, cb:cb + 1], scale=1.0)
                        else:
                            nc.vector.tensor_scalar_add(
                                o_sb[:, c4, :], pw[:], bw_t[:, cb:cb + 1])
                    nc.sync.dma_start(
                        wfT_d[cq * 1024 + g * 512:cq * 1024 + (g + 1) * 512, :]
                        .rearrange("(b p) j -> p b j", p=128), o_sb[:])
    nc.finalize()
    return nc


def _build_neff2():
    """Non-sym fallback.  PT cols J_m via fp8 DoubleRow, then out[:, J_m]."""
    nc = bacc.Bacc()
    ap_d = nc.dram_tensor("ap", [128, 8 * 16 * 2 * 512], FP8, kind="ExternalInput")
    aot_d = nc.dram_tensor("aot", [128, 16 * 2 * 512], FP8, kind="ExternalInput")
    sp_d = nc.dram_tensor("sp", [128, 8 * 4 * 512], BF16, kind="ExternalInput")
    wfp_d = nc.dram_tensor("wfp", [128, 8 * 32 * 512], BF16, kind="ExternalInput")
    wtp_d = nc.dram_tensor("wtp", [128, 8 * 4 * 512], F32, kind="ExternalInput")
    out_d = nc.dram_tensor("outc", [C, JB], F32, kind="ExternalOutput")

    NIS = 8
    NKBB = 16
    NCS = 8

    with tile.TileContext(nc) as tc:
        with tc.tile_pool(name="const", bufs=1) as constp, \
             tc.tile_pool(name="ga", bufs=2) as gap, \
             tc.tile_pool(name="gs", bufs=2) as gsp, \
             tc.tile_pool(name="wfpool", bufs=2) as wfpool, \
             tc.tile_pool(name="wtpool", bufs=2) as wtpool, \
             tc.tile_pool(name="oo", bufs=2) as oop:
            aot_t = constp.tile([128, NKBB, 2, 512], FP8)
            pt_t = constp.tile([128, 32, 512], BF16)

            with tc.tile_pool(name="psg", bufs=8, space=bass.MemorySpace.PSUM) as psgp:
              for isup in range(NIS):
                a_t = gap.tile([128, NKBB, 2, 512], FP8, tag="a_t")
                if isup == 0:
                    for q in range(4):
                        nc.sync.dma_start(
                            aot_t[:, q * 4:(q + 1) * 4, :, :],
                            aot_d[:, q * 4096:(q + 1) * 4096]
                            .rearrange("p (k h j) -> p k h j", k=4, h=2))
                        nc.sync.dma_start(
                            a_t[:, q * 4:(q + 1) * 4, :, :],
                            ap_d[:, q * 4096:(q + 1) * 4096]
                            .rearrange("p (k h i) -> p k h i", k=4, h=2))
                else:
                    for hf in range(2):
                        nc.sync.dma_start(
                            a_t[:, hf * 8:(hf + 1) * 8, :, :],
                            ap_d[:, isup * 16384 + hf * 8192:
                                 isup * 16384 + (hf + 1) * 8192]
                            .rearrange("p (k h i) -> p k h i", k=8, h=2))
                s_t = gsp.tile([128, 4, 512], BF16, tag="s_t")
                nc.sync.dma_start(
                    s_t[:],
                    sp_d[:, isup * 2048:(isup + 1) * 2048]
                    .rearrange("p (b j) -> p b j", b=4))
                psg = [psgp.tile([128, 512], F32, tag="psg", name=f"psg{i}")
                       for i in range(4)]
                for kbb in range(NKBB):
                    for ib4 in range(4):
                        nc.tensor.matmul(
                            psg[ib4][:],
                            a_t[:, kbb, :, ib4 * 128:(ib4 + 1) * 128],
                            aot_t[:, kbb, :, :],
                            start=(kbb == 0), stop=(kbb == NKBB - 1),
                            perf_mode=DR)
                for ib4 in range(4):
                    nc.vector.tensor_mul(
                        pt_t[:, isup * 4 + ib4, :], psg[ib4][:], s_t[:, ib4, :])

            with tc.tile_pool(name="pso", bufs=8, space=bass.MemorySpace.PSUM) as psop:
              for csup in range(NCS):
                wf_t = wfpool.tile([128, 32, 512], BF16, tag="wf_t")
                nc.sync.dma_start(
                    wf_t[:],
                    wfp_d[:, csup * 16384:(csup + 1) * 16384]
                    .rearrange("p (b c) -> p b c", b=32))
                wt_t = wtpool.tile([128, 4, 512], F32, tag="wt_t")
                nc.sync.dma_start(
                    wt_t[:],
                    wtp_d[:, csup * 2048:(csup + 1) * 2048]
                    .rearrange("p (b j) -> p b j", b=4))
                pso = [psop.tile([128, 512], F32, tag="pso", name=f"pso{i}")
                       for i in range(4)]
                for ib in range(32):
                    for cb in range(4):
                        nc.tensor.matmul(
                            pso[cb][:],
                            wf_t[:, ib, cb * 128:(cb + 1) * 128],
                            pt_t[:, ib, :],
                            start=(ib == 0), stop=(ib == 31))
                for half in range(2):
                    o_sb = oop.tile([128, 2, 512], F32, tag="o_sb")
                    for c2 in range(2):
                        cb = half * 2 + c2
                        nc.vector.tensor_mul(
                            o_sb[:, c2, :], pso[cb][:], wt_t[:, cb, :])
                    nc.sync.dma_start(
                        out_d[csup * 512 + half * 256:
                              csup * 512 + (half + 1) * 256, :]
                        .rearrange("(b p) j -> p b j", p=128), o_sb[:])
    nc.finalize()
    return nc


# ---- host-side packing helpers ----

def _pack_m2_bw(lw, lb, W):
    M2 = (lw.T @ W).astype(np.float32)          # [F_RAW, C]
    bw = (lb.astype(np.float64) @ W.astype(np.float64)).astype(np.float32)
    m2p = np.ascontiguousarray(
        M2.reshape(4, 128, 4, 2, 512).transpose(1, 2, 3, 0, 4).reshape(128, -1)
        .astype(BF))
    return m2p, bw


def _pack_nfT(nf, m):
    nfT = nf[m * JB:(m + 1) * JB, :].T  # [F_RAW, JB]
    return np.ascontiguousarray(
        nfT.reshape(4, 128, JB).transpose(1, 0, 2).reshape(128, -1).astype(BF))


def _pack_a_fp8(A):
    # (p, isup, kbb, h, i) with k = kbb*256 + h*128 + p
    a8 = A.astype(F8)
    return np.ascontiguousarray(
        a8.reshape(16, 2, 128, 8, 512).transpose(2, 3, 0, 1, 4).reshape(128, -1))


def _pack_cols_kh(X, dtype):
    # X [N, JB] -> (p, kbb, h, j) with k = kbb*256 + h*128 + p
    return np.ascontiguousarray(
        X.astype(dtype).reshape(16, 2, 128, JB).transpose(2, 0, 1, 3).reshape(128, -1))


def _pack_rows_sup(X, dtype, nsup, nb):
    # X [N, JB] -> (p, sup, b, j) with row = sup*512 + b*128 + p
    return np.ascontiguousarray(
        X.astype(dtype).reshape(nsup, nb, 128, -1).transpose(2, 0, 1, 3).reshape(128, -1))


def _pack_rows128(X):
    # X [nb*128, F] -> (p, b, f) with row = b*128 + p
    nb = X.shape[0] // 128
    return np.ascontiguousarray(
        X.reshape(nb, 128, -1).transpose(1, 0, 2).reshape(128, -1).astype(BF))


def _pack_c32(v):
    # v [4096] -> [128, 32] with c = cb*128 + p
    return np.ascontiguousarray(v.reshape(32, 128).T)


_NCS = {}


def _get_nc(name):
    if name not in _NCS:
        if name.startswith("no"):
            _NCS[name] = _build_neffo(int(name[2:]))
        elif name.startswith("ng"):
            kb0, kbp = name[2:].split("_")
            _NCS[name] = _build_neffg(int(kb0), int(kbp))
        else:
            _NCS[name] = {"n1": _build_neff1, "n2": _build_neff2}[name]()
    return _NCS[name]


def _ensure_trace_hook():
    """Best-effort NTFF profiling shim (test harness only; grading runs
    without tracing)."""
    try:
        from antenv.axon_hooks import get_axon_ntff_profile_hook
        return get_axon_ntff_profile_hook() is not None
    except ImportError:
        pass
    try:
        import types
        if "/root/.axon_site" not in sys.path:
            sys.path.insert(0, "/root/.axon_site")
        from trn_agent_boot.trn_boot import _ntff_profile_via_ctypes
        hook = _ntff_profile_via_ctypes("/opt/axon/libaxon_pjrt.so")
        if hook is None:
            return False
        import antenv
        mod = types.ModuleType("antenv.axon_hooks")
        mod.get_axon_ntff_profile_hook = lambda: hook
        mod.set_axon_ntff_profile_hook = lambda h: None
        sys.modules["antenv.axon_hooks"] = mod
        antenv.axon_hooks = mod
        from concourse import bass_utils as _bu
        _bu.upload_artifacts = lambda tmpdir: ""
        return True
    except Exception:
        return False


def _run(nc, in_maps, cores, trace, tag):
    if trace:
        try:
            r = run_bass_kernel_spmd(nc, in_maps, cores, trace=True)
            LAST_EXEC[tag] = r.exec_time_ns
            LAST_RESULTS[tag] = r
            return r
        except Exception as e:
            print(f"trace run failed ({e!r}); retrying without trace")
    return run_bass_kernel_spmd(nc, in_maps, cores)


def kernel(node_features, adjacency_matrix, mask_father, neighbor_count,
           mask_hadamard, linear_w, linear_b, weight):
    trace = bool(int(os.environ.get("BASS_KERNEL_TRACE", "0"))) and _ensure_trace_hook()
    cores = list(range(M))

    nf = np.ascontiguousarray(np.asarray(node_features, dtype=np.float32))
    A = np.ascontiguousarray(np.asarray(adjacency_matrix, dtype=np.float32))
    Ao = np.ascontiguousarray(np.asarray(mask_father, dtype=np.float32)[:, 0, :])
    S = np.ascontiguousarray(np.asarray(mask_hadamard, dtype=np.float32)[:, 0, :])
    ncnt = np.asarray(neighbor_count, dtype=np.float32)
    lw = np.asarray(linear_w, dtype=np.float32)
    lb = np.asarray(linear_b, dtype=np.float32)
    W = np.ascontiguousarray(np.asarray(weight, dtype=np.float32))

    inv2 = (1.0 / np.square(ncnt.astype(np.float64)))[:, 0].astype(np.float32)
    m2p, bw = _pack_m2_bw(lw, lb, W)

    # mask_father == adjacency makes G = A^T A symmetric; the sym path
    # computes only 5/8 of G per core in launch 1 and mirrors on the host.
    sym = np.array_equal(Ao, A)

    if sym:
        # ---- launch 1: G blocks only ----
        a8 = A.astype(F8)
        # per-core contraction pruning: drop k-rows with A[k, C_m] == 0
        # (zero moving row => zero contribution to every block).  Shared-
        # moving mode (KBp=0): one permutation for all slots, the t0 tile
        # doubles as every moving operand — least DMA, which wins when the
        # device power-throttle squeezes bandwidth.
        pi0s = [np.nonzero(A[:, m * 512:(m + 1) * 512].any(axis=1))[0]
                for m in range(M)]
        KB0 = max(1, max((len(p) + 255) // 256 for p in pi0s))

        def _slotpack(X):
            kbx = X.shape[0] // 256
            return X.reshape(kbx, 2, 128, 512).transpose(2, 0, 1, 3) \
                    .reshape(128, -1)

        in1 = []
        for m in range(M):
            sl = slice(m * 512, (m + 1) * 512)
            pi = pi0s[m]
            stats = []
            a0 = np.zeros((KB0 * 256, 512), dtype=F8)
            a0[:len(pi)] = a8[pi, sl]
            stats.append(a0)
            for t in range(1, 5):
                bi = (m + t) % 8
                cols = np.arange(bi * 512, (bi + 1) * 512)
                if t == 4 and m >= 4:
                    cols = np.concatenate([cols[256:], cols[:256]])
                sb = np.zeros((KB0 * 256, 512), dtype=F8)
                sb[:len(pi)] = a8[np.ix_(pi, cols)]
                stats.append(sb)
            in1.append({"apm": np.ascontiguousarray(
                np.concatenate([_slotpack(s) for s in stats], axis=1))})
        r1 = _run(_get_nc(f"ng{KB0}_0"), in1, cores, trace, "neff1")

        # assemble full G from the pieces + symmetry, mask with S
        Gf = np.empty((N, N), dtype=np.float32)
        g1s = [r1.results[m]["g1"] for m in range(M)]
        for m in range(M):
            # diag block from the symmetric-recursion pieces
            D = np.empty((512, 512), dtype=np.float32)
            D[:, 0:256] = g1s[m][0:512, 0:256]
            D[0:256, 256:512] = g1s[m][256:512, 0:256].T
            D[256:512, 256:384] = g1s[m][0:256, 256:384]
            D[256:384, 384:512] = g1s[m][128:256, 256:384].T
            D[384:512, 384:512] = g1s[m][0:128, 384:512]
            Gf[m * 512:(m + 1) * 512, m * 512:(m + 1) * 512] = D
            for t in range(1, 4):
                bi = (m + t) % 8
                Gf[bi * 512:(bi + 1) * 512, m * 512:(m + 1) * 512] = \
                    g1s[m][t * 512:(t + 1) * 512, :]
        for m in range(M):
            for d in (5, 6, 7):
                bi = (m + d) % 8
                tp = (m - bi) % 8
                Gf[bi * 512:(bi + 1) * 512, m * 512:(m + 1) * 512] = \
                    g1s[bi][tp * 512:(tp + 1) * 512, :].T
        # pair blocks {p, p+4}: four 256x256 quadrants split across the pair
        for p in range(4):
            q = p + 4
            B = np.empty((512, 512), dtype=np.float32)
            B[0:256, 0:256] = g1s[p][2048:2304, 0:256]
            B[256:512, 256:512] = g1s[p][2304:2560, 0:256]
            B[0:256, 256:512] = g1s[q][2048:2304, 0:256].T
            B[256:512, 0:256] = g1s[q][2304:2560, 0:256].T
            Gf[q * 512:(q + 1) * 512, p * 512:(p + 1) * 512] = B
            Gf[p * 512:(p + 1) * 512, q * 512:(q + 1) * 512] = B.T
        pt = Gf * S  # PT[i, j] (G symmetric)

        # ---- launch 2: factored O phase ----
        # drop PT rows that are all-zero for each core's columns (exact);
        # B = max block count across cores keeps the NEFF uniform (SPMD)
        bwe = np.zeros((128, C), dtype=BF)
        bwe[0, :] = bw.astype(BF)
        bvip = np.ascontiguousarray(np.concatenate(
            [_pack_c32((bw.astype(np.float64) * inv2).astype(np.float32)),
             _pack_c32(inv2)], axis=1))
        parts = []
        for m in range(M):
            sl = slice(m * JB, (m + 1) * JB)
            ptm = np.ascontiguousarray(pt[:, sl])
            rs = ptm.sum(axis=0)
            nz = np.nonzero(ptm.any(axis=1))[0]
            parts.append((ptm, rs, nz))
        B = max(1, max((len(nz) + 127) // 128 for _, _, nz in parts))
        in2 = []
        for m, (ptm, rs, nz) in enumerate(parts):
            pt_sel = np.zeros((B * 128, JB), dtype=np.float32)
            pt_sel[:len(nz)] = ptm[nz]
            nf_sel = np.zeros((B * 128, F_RAW), dtype=np.float32)
            nf_sel[:len(nz)] = nf[nz]
            q4 = np.zeros((128, 512), dtype=BF)
            q4[0, :] = rs.astype(BF)
            in2.append({
                "ptp": _pack_rows128(pt_sel),
                "nfp": _pack_rows128(nf_sel),
                "m2": m2p,
                "nfT": _pack_nfT(nf, m),
                "bwe": bwe,
                "q4": q4,
                "bvi": bvip,
            })
        r2 = _run(_get_nc(f"no{B}"), in2, cores, trace, "neff2")
    else:
        # ---- fallback: original two-launch path ----
        bwp = _pack_c32(bw)
        in1 = [{"m2": m2p, "nfT": _pack_nfT(nf, m), "bw": bwp}
               for m in range(M)]
        r1 = _run(_get_nc("n1"), in1, cores, trace, "neff1")
        wfT = np.concatenate([r1.results[m]["wft_rows"] for m in range(M)],
                             axis=1)
        wfb = np.ascontiguousarray(wfT.T)  # [N, C] bf16
        wfT32 = wfT.astype(np.float32)
        wfp = np.ascontiguousarray(
            wfb.reshape(32, 128, 8, 512).transpose(1, 2, 0, 3).reshape(128, -1))
        a_pack = _pack_a_fp8(A)
        in2 = []
        for m in range(M):
            sl = slice(m * JB, (m + 1) * JB)
            wt = wfT32[:, sl] * inv2[:, None]
            in2.append({
                "ap": a_pack,
                "aot": _pack_cols_kh(np.ascontiguousarray(Ao[:, sl]), F8),
                "sp": _pack_rows_sup(np.ascontiguousarray(S[:, sl]), BF, 8, 4),
                "wfp": wfp,
                "wtp": _pack_rows_sup(wt.astype(np.float32), np.float32, 8, 4),
            })
        r2 = _run(_get_nc("n2"), in2, cores, trace, "neff2")

    out = np.empty((C, N), dtype=np.float32)
    for m in range(M):
        out[:, m * JB:(m + 1) * JB] = r2.results[m]["outc"]
    return out


# revision 63
# speedup vs baseline: 1.1452x; 1.0086x over previous
"""Distributed Bass kernel for nn_Interaction_GraphConvolution.

Math (reference):
    x  = node_features @ linear_w.T + linear_b          [N, IN_F]
    wf = x @ weight                                     [N, C]
    G  = mask_father[:,0,:].T @ adjacency               [N, N]
    P  = G * mask_hadamard[:,0,:].T                     [N, N]
    out[c, j] = wf[j,c] * (P @ wf)[j,c] / neighbor_count[c]^2

Key factorization (sym fast path): wf = nf @ M2 + 1*bw with
M2 = lw.T @ W (host, weights only), so
    P @ wf = (P @ nf) @ M2 + (P @ 1) x bw
turning the [N,N]@[N,C] GEMM (17.2 GF/core) into two skinny GEMMs
through the rank-512 bottleneck (2.15 GF each) plus a rank-1 term.

Sharding: output columns j (node dim) split across 8 cores, 512 each.
Two SPMD launches:
  NEFF-G: core m computes its diagonal G block via symmetric recursion,
          3 full off-diagonal row-blocks, and two 256x256 quadrants of
          the shared {m, m+4} pair block of symmetric G = A^T A
          (fp8 DoubleRow, exact on 0/1).  k-rows of A that are zero over
          the core's columns are dropped host-side (exact).
  NEFF-O: core m computes Q^T = nf^T @ PT[:,J_m] (PT rows that are
          all-zero dropped host-side), then Z^T = M2^T @ Q^T + bw x rs
          (rs = PT colsums, 5th k-block), wfT[:,J_m] on-core, and
          out = Z^T * wfT * inv2.

All DRAM operands are host-packed so each DMA moves a multi-KB
contiguous line per partition; DMA issue order puts each launch's
smallest PE-unblocking input first.
"""

import os
import sys

sys.path.insert(0, "/opt/trn_rl_repo")

import numpy as np
import ml_dtypes

from concourse import bass, bacc, mybir, tile
from concourse.bass_utils import run_bass_kernel_spmd

F32 = mybir.dt.float32
BF16 = mybir.dt.bfloat16
FP8 = mybir.dt.float8e4
DR = mybir.MatmulPerfMode.DoubleRow
IDENT = mybir.ActivationFunctionType.Identity

BF = ml_dtypes.bfloat16
F8 = ml_dtypes.float8_e4m3fn

N = 4096       # nodes (== out channels C)
F_RAW = 512    # raw feature dim
IN_F = 1024    # hidden dim
C = 4096       # out channels
M = 8          # cores
JB = N // M    # 512 output columns per core

LAST_EXEC = {}
LAST_RESULTS = {}


def _build_neffg(KB0=16, KBp=16):
    """G half via symmetry.  Core m computes, for its columns C_m:
      t=0 diag block D = G[C_m, C_m] via symmetric recursion: D[:, 0:256],
          then D[256:512, 256:384] and D[384:512, 384:512] (the rest is
          mirrored on the host) — all operands are slices of one A tile.
      t=1..3 full row-blocks (m+t)%8.
      t=4 two 256x256 quadrants of the shared {m, m+4} pair block (for
          m >= 4 the stat column halves are swapped host-side so the pair
          covers all four quadrants with no duplication).
    Contraction pruning: k-rows of A that are zero over the relevant
    column sets are dropped host-side (exact).  KB0 = 256-row k-blocks for
    t=0 (A[k, C_m] != 0); KBp = k-blocks for t=1..4 (nonzero on BOTH the
    stat and moving column sets) — each t has its own permuted stat AND
    moving copy, aligned row-by-row.

    KBp == 0 selects the shared-moving mode: t=1..4 stats are packed with
    the t0 permutation (KB0 blocks) and the moving operand is the t0 tile
    itself — 6 MB less DMA, a few more matmuls; the right trade when the
    device power-throttle squeezes DMA bandwidth.

    apm [128, (KB0+4*KBx)*1024] fp8 : slot 0 = t0 operand (stat==moving);
                                      slots 1..4 = stats, (p, kbb, h, i)
    aom [128, 4*KBp*1024] fp8       : movings for t=1..4 (absent if KBp=0)
    out g1 [5*512, JB] bf16 : t0 pieces at [0:512, 0:256], [0:256,
        256:384], [0:128, 384:512]; t1-3 full rows t*512; t4 cols 0:256
    """
    KBx = KBp if KBp > 0 else KB0
    nc = bacc.Bacc()
    apm_d = nc.dram_tensor("apm", [128, (KB0 + 4 * KBx) * 1024], FP8,
                           kind="ExternalInput")
    if KBp > 0:
        aom_d = nc.dram_tensor("aom", [128, 4 * KBp * 1024], FP8,
                               kind="ExternalInput")
    g1_d = nc.dram_tensor("g1", [5 * 512, JB], BF16, kind="ExternalOutput")

    with tile.TileContext(nc) as tc:
        with tc.tile_pool(name="const", bufs=1) as constp, \
             tc.tile_pool(name="ga", bufs=3) as gap, \
             tc.tile_pool(name="gm", bufs=3) as gmp, \
             tc.tile_pool(name="io1", bufs=4) as iop, \
             tc.tile_pool(name="psg", bufs=8, space=bass.MemorySpace.PSUM) as psgp:
            aot_t = constp.tile([128, KB0, 2, 512], FP8)
            # first three chunk issues go to three different engine queues
            # so their ~0.65us issue costs run concurrently (engines idle)
            engs = [nc.sync, nc.scalar, nc.gpsimd]
            pos, ei = 0, 0
            while pos < KB0:
                nk = min(2, KB0 - pos)
                engs[ei].dma_start(
                    aot_t[:, pos:pos + nk, :, :],
                    apm_d[:, pos * 1024:(pos + nk) * 1024]
                    .rearrange("p (k h j) -> p k h j", k=nk, h=2))
                ei = (ei + 1) % 3
                pos += nk

            # t0: P1 = D[:, 0:256], P2a = D[256:512, 256:384],
            #     P2b = D[384:512, 384:512]
            psg = [psgp.tile([128, 512], F32, tag="psg", name=f"psg0_{i}")
                   for i in range(4)]
            ps2 = [psgp.tile([128, 512], F32, tag="psg", name=f"psg0b_{i}")
                   for i in range(3)]
            for kbb in range(KB0):
                st, sp = (kbb == 0), (kbb == KB0 - 1)
                for ib4 in range(4):
                    nc.tensor.matmul(
                        psg[ib4][:, 0:256],
                        aot_t[:, kbb, :, ib4 * 128:(ib4 + 1) * 128],
                        aot_t[:, kbb, :, 0:256],
                        start=st, stop=sp, perf_mode=DR)
                for i2 in range(2):
                    nc.tensor.matmul(
                        ps2[i2][:, 0:128],
                        aot_t[:, kbb, :, (2 + i2) * 128:(3 + i2) * 128],
                        aot_t[:, kbb, :, 256:384],
                        start=st, stop=sp, perf_mode=DR)
                nc.tensor.matmul(
                    ps2[2][:, 0:128],
                    aot_t[:, kbb, :, 384:512],
                    aot_t[:, kbb, :, 384:512],
                    start=st, stop=sp, perf_mode=DR)
            og1 = iop.tile([128, 4, 256], BF16, tag="og256")
            for ib4 in range(4):
                if ib4 % 2 == 0:
                    nc.scalar.activation(og1[:, ib4, :], psg[ib4][:, 0:256],
                                         IDENT, bias=0.0, scale=1.0)
                else:
                    nc.vector.tensor_copy(og1[:, ib4, :], psg[ib4][:, 0:256])
            nc.sync.dma_start(
                g1_d[0:512, 0:256].rearrange("(b p) j -> p b j", p=128),
                og1[:])
            og2 = iop.tile([128, 3, 128], BF16, tag="og128")
            for i2 in range(3):
                if i2 % 2 == 0:
                    nc.scalar.activation(og2[:, i2, :], ps2[i2][:, 0:128],
                                         IDENT, bias=0.0, scale=1.0)
                else:
                    nc.vector.tensor_copy(og2[:, i2, :], ps2[i2][:, 0:128])
            nc.sync.dma_start(
                g1_d[0:256, 256:384].rearrange("(b p) j -> p b j", p=128),
                og2[:, 0:2, :])
            nc.sync.dma_start(
                g1_d[0:128, 384:512].rearrange("(b p) j -> p b j", p=128),
                og2[:, 2:3, :])

            # t1..4: stat from apm slot t; moving from aom slot t-1, or the
            # t0 tile in shared-moving mode
            for t in range(1, 5):
                a_t = gap.tile([128, KBx, 2, 512], FP8, tag="a_t")
                sbase = (KB0 + (t - 1) * KBx) * 1024
                mbase = (t - 1) * KBx * 1024
                ao_t = aot_t if KBp == 0 else \
                    gmp.tile([128, KBx, 2, 512], FP8, tag="ao_t")
                pos = 0
                while pos < KBx:
                    nk = min(8, KBx - pos)
                    nc.sync.dma_start(
                        a_t[:, pos:pos + nk, :, :],
                        apm_d[:, sbase + pos * 1024:sbase + (pos + nk) * 1024]
                        .rearrange("p (k h i) -> p k h i", k=nk, h=2))
                    if KBp > 0:
                        nc.sync.dma_start(
                            ao_t[:, pos:pos + nk, :, :],
                            aom_d[:, mbase + pos * 1024:
                                  mbase + (pos + nk) * 1024]
                            .rearrange("p (k h i) -> p k h i", k=nk, h=2))
                    pos += nk
                psgt = [psgp.tile([128, 512], F32, tag="psg",
                                  name=f"psg{t}_{i}") for i in range(4)]
                half = 512 if t < 4 else 256
                for kbb in range(KBx):
                    for ib4 in range(4):
                        jlo = 0 if (t < 4 or ib4 < 2) else 256
                        nc.tensor.matmul(
                            psgt[ib4][:, 0:half],
                            a_t[:, kbb, :, ib4 * 128:(ib4 + 1) * 128],
                            ao_t[:, kbb, :, jlo:jlo + half],
                            start=(kbb == 0), stop=(kbb == KBx - 1),
                            perf_mode=DR)
                og = iop.tile([128, 4, half], BF16, tag=f"ogt{half}")
                for ib4 in range(4):
                    if ib4 % 2 == 0:
                        nc.scalar.activation(og[:, ib4, :],
                                             psgt[ib4][:, 0:half],
                                             IDENT, bias=0.0, scale=1.0)
                    else:
                        nc.vector.tensor_copy(og[:, ib4, :],
                                              psgt[ib4][:, 0:half])
                nc.sync.dma_start(
                    g1_d[t * 512:(t + 1) * 512, 0:half]
                    .rearrange("(b p) j -> p b j", p=128), og[:])
    nc.finalize()
    return nc


def _build_neffo(B=32):
    """Factored O phase.  B = number of 128-row i-blocks kept after the
    host drops PT rows that are all-zero for this core's columns (the
    same row permutation is applied to nf, so the contraction is exact).

    ptp [128, B*512] bf16  : PT[:, J_m] packed (p, ib, j), i = ib*128+p
    nfp [128, B*512] bf16  : nf packed (p, ib, f), i = ib*128+p
    m2  [128, 16*1024] bf16: M2 packed (p, cq, g, rb, cw), f = rb*128+p,
                             c = cq*1024 + g*512 + cw
    nfT [128, 4*512] bf16  : nf[J_m].T packed (p, rb, j), f = rb*128+p
    bwe [128, 4096] bf16   : row 0 = bw (stat for the rank-1 term; doing
                             the rank-1 on scalar/DVE/GpSimd instead was
                             measured SLOWER — GpSimd tensor ops are
                             1270 ns per [128,512] and the cross-engine
                             chain beats the 32 matmuls it saves)
    q4  [128, 512] bf16    : row 0 = rs = colsums of PT[:, J_m]
    bvi [128, 64] f32      : cols 0:32 = bw*inv2, cols 32:64 = inv2,
                             packed c = cb*128+p
    out outc [C, JB] f32
    """
    nc = bacc.Bacc()
    ptp_d = nc.dram_tensor("ptp", [128, B * 512], BF16, kind="ExternalInput")
    nfp_d = nc.dram_tensor("nfp", [128, B * 512], BF16, kind="ExternalInput")
    m2_d = nc.dram_tensor("m2", [128, 16 * 1024], BF16, kind="ExternalInput")
    nfT_d = nc.dram_tensor("nfT", [128, 4 * JB], BF16, kind="ExternalInput")
    bwe_d = nc.dram_tensor("bwe", [128, C], BF16, kind="ExternalInput")
    q4_d = nc.dram_tensor("q4", [128, 512], BF16, kind="ExternalInput")
    bvi_d = nc.dram_tensor("bvi", [128, 64], F32, kind="ExternalInput")
    out_d = nc.dram_tensor("outc", [C, JB], F32, kind="ExternalOutput")

    with tile.TileContext(nc) as tc:
        with tc.tile_pool(name="const", bufs=1) as constp, \
             tc.tile_pool(name="m2p", bufs=2) as m2p, \
             tc.tile_pool(name="wz", bufs=4) as wzp, \
             tc.tile_pool(name="oo", bufs=4) as oop, \
             tc.tile_pool(name="ps2", bufs=8, space=bass.MemorySpace.PSUM) as psp:
            ptp_t = constp.tile([128, B, 512], BF16)
            nfp_t = constp.tile([128, B, 512], BF16)
            qt_t = constp.tile([128, 5, 512], BF16)
            nfT_t = constp.tile([128, 4, JB], BF16)
            bwe_t = constp.tile([128, C], BF16)
            bvi_t = constp.tile([128, 64], F32)
            bwi_t = bvi_t[:, 0:32]
            inv_t = bvi_t[:, 32:64]

            # critical-path DMA order: Q's first 4-block chunk (1 MB) is the
            # smallest dependency that lets the PE start; the W inputs and
            # the rest of the Q stream land behind it.
            nc.sync.dma_start(bvi_t, bvi_d[:])
            m2_t0 = m2p.tile([128, 2, 4, 512], BF16, tag="m2_t", name="m2_t0")

            def w_block(m2_t, cb, c4, g):
                pwf = psp.tile([128, 512], F32, tag="pwf", bufs=2,
                               name=f"pwf{cb}")
                for rb in range(4):
                    nc.tensor.matmul(
                        pwf[:],
                        m2_t[:, g, rb, c4 * 128:(c4 + 1) * 128],
                        nfT_t[:, rb, :],
                        start=(rb == 0), stop=(rb == 3))
                wsb = wzp.tile([128, 512], F32, tag="wsb", bufs=8,
                               name=f"wsb{cb}")
                nc.scalar.activation(
                    wsb[:], pwf[:], IDENT,
                    bias=bwi_t[:, cb:cb + 1], scale=inv_t[:, cb:cb + 1])
                return wsb

            def z_block(m2_t, wsb, osb, cb, c4, g):
                pz = psp.tile([128, 512], F32, tag="pz", bufs=2,
                              name=f"pz{cb}")
                for rb in range(4):
                    nc.tensor.matmul(
                        pz[:],
                        m2_t[:, g, rb, c4 * 128:(c4 + 1) * 128],
                        qt_t[:, rb, :],
                        start=(rb == 0), stop=False)
                nc.tensor.matmul(
                    pz[:],
                    bwe_t[:, cb * 128:(cb + 1) * 128],
                    qt_t[:, 4, :],
                    start=False, stop=True)
                nc.vector.tensor_mul(osb[:, c4, :], pz[:], wsb[:])

            def q_dma(pos, nb):
                nc.sync.dma_start(
                    nfp_t[:, pos:pos + nb, :],
                    nfp_d[:, pos * 512:(pos + nb) * 512]
                    .rearrange("p (b f) -> p b f", b=nb))
                nc.sync.dma_start(
                    ptp_t[:, pos:pos + nb, :],
                    ptp_d[:, pos * 512:(pos + nb) * 512]
                    .rearrange("p (b j) -> p b j", b=nb))

            qps = [psp.tile([128, 512], F32, tag="qps", name=f"qps{i}", bufs=4)
                   for i in range(4)]

            def q_mms(lo, hi):
                # Q^T[f, j] += sum_i nf[i, f] * PT[i, j], i-blocks [lo, hi)
                for ib in range(lo, min(hi, B)):
                    for fb in range(4):
                        nc.tensor.matmul(
                            qps[fb][:],
                            nfp_t[:, ib, fb * 128:(fb + 1) * 128],
                            ptp_t[:, ib, :],
                            start=(ib == 0), stop=(ib == B - 1))

            # interleave: Q chunks first (smallest PE-unblocking data), W g0
            # and g1 slotted in as their inputs land.  The W accumulation
            # groups use separate PSUM banks, so they nest freely inside
            # Q's open accumulation chain.
            wsb0 = [[None] * 4 for _ in range(2)]
            # issue the four critical head transfers from four different
            # engine queues so their ~0.65us issue costs run concurrently
            # (all engines are otherwise idle at the head)
            n0 = min(2, B)
            nc.sync.dma_start(
                nfp_t[:, 0:n0, :],
                nfp_d[:, 0:n0 * 512].rearrange("p (b f) -> p b f", b=n0))
            nc.scalar.dma_start(
                ptp_t[:, 0:n0, :],
                ptp_d[:, 0:n0 * 512].rearrange("p (b j) -> p b j", b=n0))
            if B > 2:
                n1 = min(2, B - 2)
                nc.gpsimd.dma_start(
                    nfp_t[:, 2:2 + n1, :],
                    nfp_d[:, 1024:(2 + n1) * 512]
                    .rearrange("p (b f) -> p b f", b=n1))
                nc.sync.dma_start(
                    ptp_t[:, 2:2 + n1, :],
                    ptp_d[:, 1024:(2 + n1) * 512]
                    .rearrange("p (b j) -> p b j", b=n1))
            nc.scalar.dma_start(bvi_t, bvi_d[:])
            q_mms(0, 4)
            nc.sync.dma_start(nfT_t[:],
                              nfT_d[:].rearrange("p (r j) -> p r j", r=4))
            nc.sync.dma_start(
                m2_t0[:, 0],
                m2_d[:, 0:2048].rearrange("p (r c) -> p r c", r=4))
            if B > 4:
                q_dma(4, min(4, B - 4))
            for c4 in range(4):
                wsb0[0][c4] = w_block(m2_t0, c4, c4, 0)
            nc.sync.dma_start(
                m2_t0[:, 1],
                m2_d[:, 2048:4096].rearrange("p (r c) -> p r c", r=4))
            q_mms(4, 8)
            if B > 8:
                q_dma(8, min(8, B - 8))
            for c4 in range(4):
                wsb0[1][c4] = w_block(m2_t0, 4 + c4, c4, 1)
            q_mms(8, 16)
            pos = 16
            while pos < B:
                nb = min(8, B - pos)
                q_dma(pos, nb)
                q_mms(pos, pos + nb)
                pos += nb
            nc.sync.dma_start(qt_t[:, 4, :], q4_d[:])
            nc.sync.dma_start(bwe_t[:], bwe_d[:])
            for fb in range(4):
                if fb % 2 == 0:
                    nc.scalar.activation(qt_t[:, fb, :], qps[fb][:],
                                         IDENT, bias=0.0, scale=1.0)
                else:
                    nc.vector.tensor_copy(qt_t[:, fb, :], qps[fb][:])

            # Z for cq0 using the stored wsb tiles
            for g in range(2):
                osb = oop.tile([128, 4, 512], F32, tag="osb", name=f"osb0{g}")
                for c4 in range(4):
                    z_block(m2_t0, wsb0[g][c4], osb, g * 4 + c4, c4, g)
                    if c4 % 2 == 1:
                        base = g * 512 + (c4 - 1) * 128
                        nc.sync.dma_start(
                            out_d[base:base + 256, :]
                            .rearrange("(b p) j -> p b j", p=128),
                            osb[:, c4 - 1:c4 + 1, :])

            # remaining c-quarters: W+Z interleaved per 128-c block
            for cq in range(1, 4):
                m2_t = m2p.tile([128, 2, 4, 512], BF16, tag="m2_t",
                                name=f"m2_t{cq}")
                for g in range(2):
                    nc.sync.dma_start(
                        m2_t[:, g],
                        m2_d[:, cq * 4096 + g * 2048:cq * 4096 + (g + 1) * 2048]
                        .rearrange("p (r c) -> p r c", r=4))
                for g in range(2):
                    osb = oop.tile([128, 4, 512], F32, tag="osb",
                                   name=f"osb{cq}{g}")
                    for c4 in range(4):
                        cb = cq * 8 + g * 4 + c4
                        wsb = w_block(m2_t, cb, c4, g)
                        z_block(m2_t, wsb, osb, cb, c4, g)
                        if c4 % 2 == 1:
                            base = cq * 1024 + g * 512 + (c4 - 1) * 128
                            nc.sync.dma_start(
                                out_d[base:base + 256, :]
                                .rearrange("(b p) j -> p b j", p=128),
                                osb[:, c4 - 1:c4 + 1, :])
    nc.finalize()
    return nc


def _build_neff1():
    """Non-sym fallback.  Per core: wfT[:, J_m] = M2.T @ nf[J_m].T + bw."""
    nc = bacc.Bacc()
    m2_d = nc.dram_tensor("m2", [128, 16 * 1024], BF16, kind="ExternalInput")
    nfT_d = nc.dram_tensor("nfT", [128, 4 * JB], BF16, kind="ExternalInput")
    bw_d = nc.dram_tensor("bw", [128, 32], F32, kind="ExternalInput")
    wfT_d = nc.dram_tensor("wft_rows", [C, JB], BF16, kind="ExternalOutput")

    NRB = 4
    NCQ = 4

    with tile.TileContext(nc) as tc:
        with tc.tile_pool(name="const", bufs=1) as constp, \
             tc.tile_pool(name="m2p", bufs=2) as m2p, \
             tc.tile_pool(name="ps1", bufs=8, space=bass.MemorySpace.PSUM) as psp, \
             tc.tile_pool(name="io1", bufs=4) as iop:
            nfT_t = constp.tile([128, NRB, JB], BF16)
            nc.sync.dma_start(
                nfT_t[:], nfT_d[:].rearrange("p (r j) -> p r j", r=NRB))
            bw_t = constp.tile([128, 32], F32)
            nc.sync.dma_start(bw_t[:], bw_d[:])

            for cq in range(NCQ):
                m2_t = m2p.tile([128, 2, NRB, 512], BF16, tag="m2_t")
                for g in range(2):
                    nc.sync.dma_start(
                        m2_t[:, g, :, :],
                        m2_d[:, cq * 4096 + g * 2048:cq * 4096 + (g + 1) * 2048]
                        .rearrange("p (r c) -> p r c", r=NRB))
                for g in range(2):
                    o_sb = iop.tile([128, 4, 512], BF16, tag="o_sb")
                    for c4 in range(4):
                        cb = cq * 8 + g * 4 + c4
                        pw = psp.tile([128, 512], F32, tag="pw")
                        for rb in range(NRB):
                            nc.tensor.matmul(
                                pw[:],
                                m2_t[:, g, rb, c4 * 128:(c4 + 1) * 128],
                                nfT_t[:, rb, :],
                                start=(rb == 0), stop=(rb == NRB - 1))
                        if c4 % 2 == 0:
                            nc.scalar.activation(
                                o_sb[:, c4, :], pw[:], IDENT,
                                bias=bw_t[:, cb:cb + 1], scale=1.0)
                        else:
                            nc.vector.tensor_scalar_add(
                                o_sb[:, c4, :], pw[:], bw_t[:, cb:cb + 1])
                    nc.sync.dma_start(
                        wfT_d[cq * 1024 + g * 512:cq * 1024 + (g + 1) * 512, :]
                        .rearrange("(b p) j -> p b j", p=128), o_sb[:])
    nc.finalize()
    return nc


def _build_neff2():
    """Non-sym fallback.  PT cols J_m via fp8 DoubleRow, then out[:, J_m]."""
    nc = bacc.Bacc()
    ap_d = nc.dram_tensor("ap", [128, 8 * 16 * 2 * 512], FP8, kind="ExternalInput")
    aot_d = nc.dram_tensor("aot", [128, 16 * 2 * 512], FP8, kind="ExternalInput")
    sp_d = nc.dram_tensor("sp", [128, 8 * 4 * 512], BF16, kind="ExternalInput")
    wfp_d = nc.dram_tensor("wfp", [128, 8 * 32 * 512], BF16, kind="ExternalInput")
    wtp_d = nc.dram_tensor("wtp", [128, 8 * 4 * 512], F32, kind="ExternalInput")
    out_d = nc.dram_tensor("outc", [C, JB], F32, kind="ExternalOutput")

    NIS = 8
    NKBB = 16
    NCS = 8

    with tile.TileContext(nc) as tc:
        with tc.tile_pool(name="const", bufs=1) as constp, \
             tc.tile_pool(name="ga", bufs=2) as gap, \
             tc.tile_pool(name="gs", bufs=2) as gsp, \
             tc.tile_pool(name="wfpool", bufs=2) as wfpool, \
             tc.tile_pool(name="wtpool", bufs=2) as wtpool, \
             tc.tile_pool(name="oo", bufs=2) as oop:
            aot_t = constp.tile([128, NKBB, 2, 512], FP8)
            pt_t = constp.tile([128, 32, 512], BF16)

            with tc.tile_pool(name="psg", bufs=8, space=bass.MemorySpace.PSUM) as psgp:
              for isup in range(NIS):
                a_t = gap.tile([128, NKBB, 2, 512], FP8, tag="a_t")
                if isup == 0:
                    for q in range(4):
                        nc.sync.dma_start(
                            aot_t[:, q * 4:(q + 1) * 4, :, :],
                            aot_d[:, q * 4096:(q + 1) * 4096]
                            .rearrange("p (k h j) -> p k h j", k=4, h=2))
                        nc.sync.dma_start(
                            a_t[:, q * 4:(q + 1) * 4, :, :],
                            ap_d[:, q * 4096:(q + 1) * 4096]
                            .rearrange("p (k h i) -> p k h i", k=4, h=2))
                else:
                    for hf in range(2):
                        nc.sync.dma_start(
                            a_t[:, hf * 8:(hf + 1) * 8, :, :],
                            ap_d[:, isup * 16384 + hf * 8192:
                                 isup * 16384 + (hf + 1) * 8192]
                            .rearrange("p (k h i) -> p k h i", k=8, h=2))
                s_t = gsp.tile([128, 4, 512], BF16, tag="s_t")
                nc.sync.dma_start(
                    s_t[:],
                    sp_d[:, isup * 2048:(isup + 1) * 2048]
                    .rearrange("p (b j) -> p b j", b=4))
                psg = [psgp.tile([128, 512], F32, tag="psg", name=f"psg{i}")
                       for i in range(4)]
                for kbb in range(NKBB):
                    for ib4 in range(4):
                        nc.tensor.matmul(
                            psg[ib4][:],
                            a_t[:, kbb, :, ib4 * 128:(ib4 + 1) * 128],
                            aot_t[:, kbb, :, :],
                            start=(kbb == 0), stop=(kbb == NKBB - 1),
                            perf_mode=DR)
                for ib4 in range(4):
                    nc.vector.tensor_mul(
                        pt_t[:, isup * 4 + ib4, :], psg[ib4][:], s_t[:, ib4, :])

            with tc.tile_pool(name="pso", bufs=8, space=bass.MemorySpace.PSUM) as psop:
              for csup in range(NCS):
                wf_t = wfpool.tile([128, 32, 512], BF16, tag="wf_t")
                nc.sync.dma_start(
                    wf_t[:],
                    wfp_d[:, csup * 16384:(csup + 1) * 16384]
                    .rearrange("p (b c) -> p b c", b=32))
                wt_t = wtpool.tile([128, 4, 512], F32, tag="wt_t")
                nc.sync.dma_start(
                    wt_t[:],
                    wtp_d[:, csup * 2048:(csup + 1) * 2048]
                    .rearrange("p (b j) -> p b j", b=4))
                pso = [psop.tile([128, 512], F32, tag="pso", name=f"pso{i}")
                       for i in range(4)]
                for ib in range(32):
                    for cb in range(4):
                        nc.tensor.matmul(
                            pso[cb][:],
                            wf_t[:, ib, cb * 128:(cb + 1) * 128],
                            pt_t[:, ib, :],
                            start=(ib == 0), stop=(ib == 31))
                for half in range(2):
                    o_sb = oop.tile([128, 2, 512], F32, tag="o_sb")
                    for c2 in range(2):
                        cb = half * 2 + c2
                        nc.vector.tensor_mul(
                            o_sb[:, c2, :], pso[cb][:], wt_t[:, cb, :])
                    nc.sync.dma_start(
                        out_d[csup * 512 + half * 256:
                              csup * 512 + (half + 1) * 256, :]
                        .rearrange("(b p) j -> p b j", p=128), o_sb[:])
    nc.finalize()
    return nc


# ---- host-side packing helpers ----

def _pack_m2_bw(lw, lb, W):
    M2 = (lw.T @ W).astype(np.float32)          # [F_RAW, C]
    bw = (lb.astype(np.float64) @ W.astype(np.float64)).astype(np.float32)
    m2p = np.ascontiguousarray(
        M2.reshape(4, 128, 4, 2, 512).transpose(1, 2, 3, 0, 4).reshape(128, -1)
        .astype(BF))
    return m2p, bw


def _pack_nfT(nf, m):
    nfT = nf[m * JB:(m + 1) * JB, :].T  # [F_RAW, JB]
    return np.ascontiguousarray(
        nfT.reshape(4, 128, JB).transpose(1, 0, 2).reshape(128, -1).astype(BF))


def _pack_a_fp8(A):
    # (p, isup, kbb, h, i) with k = kbb*256 + h*128 + p
    a8 = A.astype(F8)
    return np.ascontiguousarray(
        a8.reshape(16, 2, 128, 8, 512).transpose(2, 3, 0, 1, 4).reshape(128, -1))


def _pack_cols_kh(X, dtype):
    # X [N, JB] -> (p, kbb, h, j) with k = kbb*256 + h*128 + p
    return np.ascontiguousarray(
        X.astype(dtype).reshape(16, 2, 128, JB).transpose(2, 0, 1, 3).reshape(128, -1))


def _pack_rows_sup(X, dtype, nsup, nb):
    # X [N, JB] -> (p, sup, b, j) with row = sup*512 + b*128 + p
    return np.ascontiguousarray(
        X.astype(dtype).reshape(nsup, nb, 128, -1).transpose(2, 0, 1, 3).reshape(128, -1))


def _pack_rows128(X):
    # X [nb*128, F] -> (p, b, f) with row = b*128 + p
    nb = X.shape[0] // 128
    return np.ascontiguousarray(
        X.reshape(nb, 128, -1).transpose(1, 0, 2).reshape(128, -1).astype(BF))


def _pack_c32(v):
    # v [4096] -> [128, 32] with c = cb*128 + p
    return np.ascontiguousarray(v.reshape(32, 128).T)


_NCS = {}


def _get_nc(name):
    if name not in _NCS:
        if name.startswith("no"):
            _NCS[name] = _build_neffo(int(name[2:]))
        elif name.startswith("ng"):
            kb0, kbp = name[2:].split("_")
            _NCS[name] = _build_neffg(int(kb0), int(kbp))
        else:
            _NCS[name] = {"n1": _build_neff1, "n2": _build_neff2}[name]()
    return _NCS[name]


def _ensure_trace_hook():
    """Best-effort NTFF profiling shim (test harness only; grading runs
    without tracing)."""
    try:
        from antenv.axon_hooks import get_axon_ntff_profile_hook
        return get_axon_ntff_profile_hook() is not None
    except ImportError:
        pass
    try:
        import types
        if "/root/.axon_site" not in sys.path:
            sys.path.insert(0, "/root/.axon_site")
        from trn_agent_boot.trn_boot import _ntff_profile_via_ctypes
        hook = _ntff_profile_via_ctypes("/opt/axon/libaxon_pjrt.so")
        if hook is None:
            return False
        import antenv
        mod = types.ModuleType("antenv.axon_hooks")
        mod.get_axon_ntff_profile_hook = lambda: hook
        mod.set_axon_ntff_profile_hook = lambda h: None
        sys.modules["antenv.axon_hooks"] = mod
        antenv.axon_hooks = mod
        from concourse import bass_utils as _bu
        _bu.upload_artifacts = lambda tmpdir: ""
        return True
    except Exception:
        return False


def _run(nc, in_maps, cores, trace, tag):
    if trace:
        try:
            r = run_bass_kernel_spmd(nc, in_maps, cores, trace=True)
            LAST_EXEC[tag] = r.exec_time_ns
            LAST_RESULTS[tag] = r
            return r
        except Exception as e:
            print(f"trace run failed ({e!r}); retrying without trace")
    return run_bass_kernel_spmd(nc, in_maps, cores)


def kernel(node_features, adjacency_matrix, mask_father, neighbor_count,
           mask_hadamard, linear_w, linear_b, weight):
    trace = bool(int(os.environ.get("BASS_KERNEL_TRACE", "0"))) and _ensure_trace_hook()
    cores = list(range(M))

    nf = np.ascontiguousarray(np.asarray(node_features, dtype=np.float32))
    A = np.ascontiguousarray(np.asarray(adjacency_matrix, dtype=np.float32))
    Ao = np.ascontiguousarray(np.asarray(mask_father, dtype=np.float32)[:, 0, :])
    S = np.ascontiguousarray(np.asarray(mask_hadamard, dtype=np.float32)[:, 0, :])
    ncnt = np.asarray(neighbor_count, dtype=np.float32)
    lw = np.asarray(linear_w, dtype=np.float32)
    lb = np.asarray(linear_b, dtype=np.float32)
    W = np.ascontiguousarray(np.asarray(weight, dtype=np.float32))

    inv2 = (1.0 / np.square(ncnt.astype(np.float64)))[:, 0].astype(np.float32)
    m2p, bw = _pack_m2_bw(lw, lb, W)

    # mask_father == adjacency makes G = A^T A symmetric; the sym path
    # computes only 5/8 of G per core in launch 1 and mirrors on the host.
    sym = np.array_equal(Ao, A)

    if sym:
        # ---- launch 1: G blocks only ----
        a8 = A.astype(F8)
        # per-core contraction pruning: drop k-rows with A[k, C_m] == 0
        # (zero moving row => zero contribution to every block).  Shared-
        # moving mode (KBp=0): one permutation for all slots, the t0 tile
        # doubles as every moving operand — least DMA, which wins when the
        # device power-throttle squeezes bandwidth.
        pi0s = [np.nonzero(A[:, m * 512:(m + 1) * 512].any(axis=1))[0]
                for m in range(M)]
        KB0 = max(1, max((len(p) + 255) // 256 for p in pi0s))

        def _slotpack(X):
            kbx = X.shape[0] // 256
            return X.reshape(kbx, 2, 128, 512).transpose(2, 0, 1, 3) \
                    .reshape(128, -1)

        in1 = []
        for m in range(M):
            sl = slice(m * 512, (m + 1) * 512)
            pi = pi0s[m]
            stats = []
            a0 = np.zeros((KB0 * 256, 512), dtype=F8)
            a0[:len(pi)] = a8[pi, sl]
            stats.append(a0)
            for t in range(1, 5):
                bi = (m + t) % 8
                cols = np.arange(bi * 512, (bi + 1) * 512)
                if t == 4 and m >= 4:
                    cols = np.concatenate([cols[256:], cols[:256]])
                sb = np.zeros((KB0 * 256, 512), dtype=F8)
                sb[:len(pi)] = a8[np.ix_(pi, cols)]
                stats.append(sb)
            in1.append({"apm": np.ascontiguousarray(
                np.concatenate([_slotpack(s) for s in stats], axis=1))})
        r1 = _run(_get_nc(f"ng{KB0}_0"), in1, cores, trace, "neff1")

        # assemble full G from the pieces + symmetry, mask with S
        Gf = np.empty((N, N), dtype=np.float32)
        g1s = [r1.results[m]["g1"] for m in range(M)]
        for m in range(M):
            # diag block from the symmetric-recursion pieces
            D = np.empty((512, 512), dtype=np.float32)
            D[:, 0:256] = g1s[m][0:512, 0:256]
            D[0:256, 256:512] = g1s[m][256:512, 0:256].T
            D[256:512, 256:384] = g1s[m][0:256, 256:384]
            D[256:384, 384:512] = g1s[m][128:256, 256:384].T
            D[384:512, 384:512] = g1s[m][0:128, 384:512]
            Gf[m * 512:(m + 1) * 512, m * 512:(m + 1) * 512] = D
            for t in range(1, 4):
                bi = (m + t) % 8
                Gf[bi * 512:(bi + 1) * 512, m * 512:(m + 1) * 512] = \
                    g1s[m][t * 512:(t + 1) * 512, :]
        for m in range(M):
            for d in (5, 6, 7):
                bi = (m + d) % 8
                tp = (m - bi) % 8
                Gf[bi * 512:(bi + 1) * 512, m * 512:(m + 1) * 512] = \
                    g1s[bi][tp * 512:(tp + 1) * 512, :].T
        # pair blocks {p, p+4}: four 256x256 quadrants split across the pair
        for p in range(4):
            q = p + 4
            B = np.empty((512, 512), dtype=np.float32)
            B[0:256, 0:256] = g1s[p][2048:2304, 0:256]
            B[256:512, 256:512] = g1s[p][2304:2560, 0:256]
            B[0:256, 256:512] = g1s[q][2048:2304, 0:256].T
            B[256:512, 0:256] = g1s[q][2304:2560, 0:256].T
            Gf[q * 512:(q + 1) * 512, p * 512:(p + 1) * 512] = B
            Gf[p * 512:(p + 1) * 512, q * 512:(q + 1) * 512] = B.T
        pt = Gf * S  # PT[i, j] (G symmetric)

        # ---- launch 2: factored O phase ----
        # drop PT rows that are all-zero for each core's columns (exact);
        # B = max block count across cores keeps the NEFF uniform (SPMD)
        bwe = np.zeros((128, C), dtype=BF)
        bwe[0, :] = bw.astype(BF)
        bvip = np.ascontiguousarray(np.concatenate(
            [_pack_c32((bw.astype(np.float64) * inv2).astype(np.float32)),
             _pack_c32(inv2)], axis=1))
        parts = []
        for m in range(M):
            sl = slice(m * JB, (m + 1) * JB)
            ptm = np.ascontiguousarray(pt[:, sl])
            rs = ptm.sum(axis=0)
            nz = np.nonzero(ptm.any(axis=1))[0]
            parts.append((ptm, rs, nz))
        B = max(1, max((len(nz) + 127) // 128 for _, _, nz in parts))
        in2 = []
        for m, (ptm, rs, nz) in enumerate(parts):
            pt_sel = np.zeros((B * 128, JB), dtype=np.float32)
            pt_sel[:len(nz)] = ptm[nz]
            nf_sel = np.zeros((B * 128, F_RAW), dtype=np.float32)
            nf_sel[:len(nz)] = nf[nz]
            q4 = np.zeros((128, 512), dtype=BF)
            q4[0, :] = rs.astype(BF)
            in2.append({
                "ptp": _pack_rows128(pt_sel),
                "nfp": _pack_rows128(nf_sel),
                "m2": m2p,
                "nfT": _pack_nfT(nf, m),
                "bwe": bwe,
                "q4": q4,
                "bvi": bvip,
            })
        r2 = _run(_get_nc(f"no{B}"), in2, cores, trace, "neff2")
    else:
        # ---- fallback: original two-launch path ----
        bwp = _pack_c32(bw)
        in1 = [{"m2": m2p, "nfT": _pack_nfT(nf, m), "bw": bwp}
               for m in range(M)]
        r1 = _run(_get_nc("n1"), in1, cores, trace, "neff1")
        wfT = np.concatenate([r1.results[m]["wft_rows"] for m in range(M)],
                             axis=1)
        wfb = np.ascontiguousarray(wfT.T)  # [N, C] bf16
        wfT32 = wfT.astype(np.float32)
        wfp = np.ascontiguousarray(
            wfb.reshape(32, 128, 8, 512).transpose(1, 2, 0, 3).reshape(128, -1))
        a_pack = _pack_a_fp8(A)
        in2 = []
        for m in range(M):
            sl = slice(m * JB, (m + 1) * JB)
            wt = wfT32[:, sl] * inv2[:, None]
            in2.append({
                "ap": a_pack,
                "aot": _pack_cols_kh(np.ascontiguousarray(Ao[:, sl]), F8),
                "sp": _pack_rows_sup(np.ascontiguousarray(S[:, sl]), BF, 8, 4),
                "wfp": wfp,
                "wtp": _pack_rows_sup(wt.astype(np.float32), np.float32, 8, 4),
            })
        r2 = _run(_get_nc("n2"), in2, cores, trace, "neff2")

    out = np.empty((C, N), dtype=np.float32)
    for m in range(M):
        out[:, m * JB:(m + 1) * JB] = r2.results[m]["outc"]
    return out
